# revision 5
# baseline (speedup 1.0000x reference)
"""EnhancedGCNWithAttention on 8 Trainium2 NeuronCores (Bass/Tile SPMD).

Strategy (node-sharded, 512 nodes per core):
  - GCN aggregation is cast as dense matmuls against a per-core count matrix
    acT[4096 src, 512 local dst] (bf16, exact small integers) built on the
    host from the edge multiset (+ self loops). Degrees, rsqrt normalization,
    and everything else arithmetic run on device.
  - A_norm @ P == diag(dinv_dst) . (acT.T @ (diag(dinv_src) . P)); the source
    scaling is applied on each core to its own P rows before AllGather, the
    dst scaling is a per-partition ACT scale on the aggregation epilogue.
  - Attention is sharded by query rows; K^T and V are AllGathered. Scores are
    computed transposed (keys on partitions) so exp row-sums ride the PE via
    a ones-vector matmul and the 1/sum lands after attn@V on tiny tensors.
"""
import sys

sys.path.insert(0, '/opt/trn_rl_repo')

import numpy as np
import ml_dtypes

N, F_IN, H, HEADS, C, E = 4096, 768, 512, 4, 32, 131072
DH = H // HEADS            # 128
NCORES = 8
S = N // NCORES            # 512 rows per core
SCALE = 1.0 / np.sqrt(DH)
EXP_SHIFT = 0.0            # softmax stability shift (scores are O(10) here)

BF16 = ml_dtypes.bfloat16

_compiled = None


def _build():
    from concourse import bacc, tile, mybir

    f32 = mybir.dt.float32
    bf16 = mybir.dt.bfloat16
    AF = mybir.ActivationFunctionType
    ALU = mybir.AluOpType

    nc = bacc.Bacc("TRN2", target_bir_lowering=False, debug=False,
                   enable_asserts=False, num_devices=NCORES)

    # ---------------- I/O ----------------
    acT_in = nc.dram_tensor("acT", [N, S], bf16, kind="ExternalInput")
    xT_in = nc.dram_tensor("xT", [F_IN, S], bf16, kind="ExternalInput")
    w1_in = nc.dram_tensor("w1", [F_IN, H], bf16, kind="ExternalInput")
    inwT_in = nc.dram_tensor("inwT", [H, 3 * H], bf16, kind="ExternalInput")
    outwT_in = nc.dram_tensor("outwT", [H, H], bf16, kind="ExternalInput")
    w2_in = nc.dram_tensor("w2", [H, H], bf16, kind="ExternalInput")
    wc_in = nc.dram_tensor("wc", [H, C], bf16, kind="ExternalInput")
    b1bc_in = nc.dram_tensor("b1bc", [128, H], f32, kind="ExternalInput")
    b2bc_in = nc.dram_tensor("b2bc", [128, H], f32, kind="ExternalInput")
    outbbc_in = nc.dram_tensor("outbbc", [128, H], f32, kind="ExternalInput")
    inbvbc_in = nc.dram_tensor("inbvbc", [128, H], f32, kind="ExternalInput")
    bccbc_in = nc.dram_tensor("bccbc", [128, C], f32, kind="ExternalInput")
    inbq_in = nc.dram_tensor("inbq", [128, HEADS], f32, kind="ExternalInput")
    inbk_in = nc.dram_tensor("inbk", [128, HEADS], f32, kind="ExternalInput")
    ident_in = nc.dram_tensor("ident", [128, 128], bf16, kind="ExternalInput")
    out_t = nc.dram_tensor("y", [S, C], f32, kind="ExternalOutput")

    with tile.TileContext(nc) as tc:
        with tc.tile_pool(name="const", bufs=1) as cpool, \
             tc.tile_pool(name="acts", bufs=1) as apool, \
             tc.tile_pool(name="stream", bufs=3) as spool, \
             tc.tile_pool(name="kv", bufs=1) as kvpool, \
             tc.tile_pool(name="work", bufs=2) as wpool, \
             tc.tile_pool(name="psAgg", bufs=4, space="PSUM") as psAgg, \
             tc.tile_pool(name="psAcc", bufs=2, space="PSUM") as psAcc, \
             tc.tile_pool(name="psAv", bufs=1, space="PSUM") as psAv, \
             tc.tile_pool(name="psSum", bufs=1, space="PSUM") as psSum, \
             tc.tile_pool(name="dram", bufs=1, space="DRAM") as dram:

            # -------- constant loads --------
            def ld(name, t_in, shape, dt, pool=cpool, view=None):
                t = pool.tile(shape, dt, tag=name)
                src = t_in.ap() if view is None else view
                nc.sync.dma_start(t[:], src)
                return t

            acT = ld("acT", acT_in, [128, N // 128, S], bf16,
                     view=acT_in.ap().rearrange("(c p) s -> p c s", p=128))
            xT = ld("xT", xT_in, [128, F_IN // 128, S], bf16,
                    view=xT_in.ap().rearrange("(c p) s -> p c s", p=128))
            w1 = ld("w1", w1_in, [128, F_IN // 128, H], bf16,
                    view=w1_in.ap().rearrange("(c p) h -> p c h", p=128))
            inwT = ld("inwT", inwT_in, [128, H // 128, 3 * H], bf16,
                      view=inwT_in.ap().rearrange("(c p) h -> p c h", p=128))
            outwT = ld("outwT", outwT_in, [128, H // 128, H], bf16,
                       view=outwT_in.ap().rearrange("(c p) h -> p c h", p=128))
            w2 = ld("w2", w2_in, [128, H // 128, H], bf16,
                    view=w2_in.ap().rearrange("(c p) h -> p c h", p=128))
            wc = ld("wc", wc_in, [128, H // 128, C], bf16,
                    view=wc_in.ap().rearrange("(c p) h -> p c h", p=128))
            b1bc = ld("b1bc", b1bc_in, [128, H], f32)
            b2bc = ld("b2bc", b2bc_in, [128, H], f32)
            outbbc = ld("outbbc", outbbc_in, [128, H], f32)
            inbvbc = ld("inbvbc", inbvbc_in, [128, H], f32)
            bccbc = ld("bccbc", bccbc_in, [128, C], f32)
            inbq = ld("inbq", inbq_in, [128, HEADS], f32)
            inbk = ld("inbk", inbk_in, [128, HEADS], f32)
            ident = ld("ident", ident_in, [128, 128], bf16)

            ones_bf = cpool.tile([128, 1], bf16, tag="ones_bf")
            nc.vector.memset(ones_bf[:], 1.0)
            ones_row = cpool.tile([1, 128], f32, tag="ones_row")
            nc.vector.memset(ones_row[:], 1.0)
            one_11 = cpool.tile([1, 1], f32, tag="one_11")
            nc.vector.memset(one_11[:], 1.0)

            # -------- degrees + dinv --------
            deg_ps = psSum.tile([1, S], f32, tag="sum", name="deg_ps")
            for c in range(N // 128):
                nc.tensor.matmul(deg_ps[:], ones_bf[:], acT[:, c, :],
                                 start=(c == 0), stop=(c == N // 128 - 1))
            rdeg = cpool.tile([1, S], f32, tag="rdeg")
            nc.vector.reciprocal(rdeg[:], deg_ps[:])
            dinv_row = cpool.tile([1, S], f32, tag="dinv_row")
            nc.scalar.activation(dinv_row[:], rdeg[:], AF.Sqrt)
            # column layout: dinv_col[p, m] = dinv_row[0, 128m + p]
            dinv_col = cpool.tile([128, S // 128], f32, tag="dinv_col")
            for m in range(S // 128):
                dc_ps = psAv.tile([128, 1], f32, tag="av", name="dc_ps")
                nc.tensor.matmul(dc_ps[:], dinv_row[:, 128 * m:128 * (m + 1)],
                                 one_11[:], start=True, stop=True)
                nc.scalar.activation(dinv_col[:, m:m + 1], dc_ps[:], AF.Copy)

            # -------- P1c = diag(dinv) . (x @ W1), AllGather --------
            ag1_in = dram.tile([S, H], bf16)
            ag1_out = dram.tile([N, H], bf16)
            for m in range(S // 128):
                ps = psAcc.tile([128, H], f32, tag="acc", name="p1_ps")
                for c in range(F_IN // 128):
                    nc.tensor.matmul(ps[:], xT[:, c, 128 * m:128 * (m + 1)],
                                     w1[:, c, :], start=(c == 0),
                                     stop=(c == F_IN // 128 - 1))
                p1c = wpool.tile([128, H], bf16, tag="p1c")
                nc.scalar.activation(p1c[:], ps[:], AF.Identity,
                                     scale=dinv_col[:, m:m + 1])
                nc.sync.dma_start(
                    ag1_in[:].rearrange("(mm p) h -> p mm h", p=128)[:, m, :],
                    p1c[:])
            nc.gpsimd.collective_compute(
                "AllGather", ALU.bypass, ins=[ag1_in.opt()],
                outs=[ag1_out.opt()], replica_groups=[list(range(NCORES))])

            # -------- conv1: h1 = relu(dinv .* (acT.T @ p1c_full) + b1) ------
            h1 = apool.tile([128, S // 128, H], bf16, tag="h1")
            h1f = apool.tile([128, S // 128, H], f32, tag="h1f")
            h1ps = [psAgg.tile([128, H], f32, tag="agg", name=f"h1ps{m}")
                    for m in range(4)]
            for c in range(N // 128):
                rhs = spool.tile([128, H], bf16, tag="agc")
                nc.sync.dma_start(
                    rhs[:], ag1_out[:].rearrange("(cc p) h -> p cc h", p=128)[:, c, :])
                for m in range(S // 128):
                    nc.tensor.matmul(h1ps[m][:], acT[:, c, 128 * m:128 * (m + 1)],
                                     rhs[:], start=(c == 0), stop=(c == N // 128 - 1))
            for m in range(S // 128):
                t = wpool.tile([128, H], f32, tag="ep1")
                nc.scalar.activation(t[:], h1ps[m][:], AF.Identity,
                                     scale=dinv_col[:, m:m + 1])
                nc.vector.tensor_tensor(t[:], t[:], b1bc[:], op=ALU.add)
                nc.vector.tensor_scalar_max(h1f[:, m, :], t[:], 0.0)
                nc.vector.tensor_copy(h1[:, m, :], h1f[:, m, :])

            # -------- T1 = h1^T --------
            t1 = apool.tile([128, H // 128, S], bf16, tag="t1")
            for c in range(H // 128):
                tp = psAcc.tile([128, S], bf16, tag="acc", name="tp_ps")
                for m in range(S // 128):
                    nc.tensor.transpose(tp[:, 128 * m:128 * (m + 1)],
                                        h1[:, m, 128 * c:128 * (c + 1)], ident[:])
                nc.scalar.activation(t1[:, c, :], tp[:], AF.Copy)

            # -------- qkv --------
            qt = apool.tile([128, HEADS, S], bf16, tag="qt")
            kt = apool.tile([128, HEADS, S], bf16, tag="kt")
            vv = apool.tile([128, S // 128, H], bf16, tag="vv")
            inbq_s = cpool.tile([128, HEADS], f32, tag="inbq_s")
            nc.vector.tensor_scalar_mul(inbq_s[:], inbq[:], SCALE)
            for h in range(HEADS):
                ps = psAcc.tile([128, S], f32, tag="acc", name="q_ps")
                for c in range(H // 128):
                    nc.tensor.matmul(ps[:], inwT[:, c, 128 * h:128 * (h + 1)],
                                     t1[:, c, :], start=(c == 0),
                                     stop=(c == H // 128 - 1))
                tq = wpool.tile([128, S], f32, tag="tq")
                nc.scalar.activation(tq[:], ps[:], AF.Identity, scale=SCALE)
                nc.vector.tensor_scalar_add(qt[:, h, :], tq[:], inbq_s[:, h:h + 1])
                ps2 = psAcc.tile([128, S], f32, tag="acc", name="k_ps")
                for c in range(H // 128):
                    nc.tensor.matmul(ps2[:], inwT[:, c, H + 128 * h:H + 128 * (h + 1)],
                                     t1[:, c, :], start=(c == 0),
                                     stop=(c == H // 128 - 1))
                nc.vector.tensor_scalar_add(kt[:, h, :], ps2[:], inbk[:, h:h + 1])
            for m in range(S // 128):
                ps = psAcc.tile([128, H], f32, tag="acc", name="v_ps")
                for c in range(H // 128):
                    nc.tensor.matmul(ps[:], t1[:, c, 128 * m:128 * (m + 1)],
                                     inwT[:, c, 2 * H:3 * H], start=(c == 0),
                                     stop=(c == H // 128 - 1))
                nc.vector.tensor_tensor(vv[:, m, :], ps[:], inbvbc[:], op=ALU.add)

            # -------- KV AllGather ([0:512] = K^T heads, [512:1024] = V rows)
            agkv_in = dram.tile([2 * H, S], bf16)
            agkv_out = dram.tile([NCORES * 2 * H, S], bf16)
            nc.sync.dma_start(
                agkv_in[:].rearrange("(z p) s -> p z s", p=128)[:, 0:HEADS, :], kt[:])
            nc.sync.dma_start(
                agkv_in[:].rearrange("(z p) s -> p z s", p=128)[:, HEADS:2 * HEADS, :],
                vv[:])
            nc.gpsimd.collective_compute(
                "AllGather", ALU.bypass, ins=[agkv_in.opt()],
                outs=[agkv_out.opt()], replica_groups=[list(range(NCORES))])

            # -------- V full load: vf[p, b, :] = V[global key 128b + p, :] ----
            agkv_v = agkv_out[:].rearrange("(cc z p) s -> p cc z s", p=128, z=8)
            vf = kvpool.tile([128, N // 128, H], bf16, tag="vf")
            for cc in range(NCORES):
                for t in range(4):
                    nc.sync.dma_start(vf[:, 4 * cc + t, :], agkv_v[:, cc, 4 + t, :])

            # -------- attention (transposed scores), per head ----------------
            oT = apool.tile([128, HEADS, S], bf16, tag="oT")
            for h in range(HEADS):
                kth = kvpool.tile([128, N], bf16, tag="kth")
                for cc in range(NCORES):
                    nc.sync.dma_start(kth[:, S * cc:S * cc + S],
                                      agkv_v[:, cc, h, :])
                av_ps = psAv.tile([128, S], f32, tag="av", name="av_ps")
                sum_ps = psSum.tile([1, S], f32, tag="sum", name="sum_ps")
                for b in range(N // 128):
                    s_ps = psAcc.tile([128, S], f32, tag="acc", name="s_ps")
                    nc.tensor.matmul(s_ps[:], kth[:, 128 * b:128 * (b + 1)],
                                     qt[:, h, :], start=True, stop=True)
                    ex = spool.tile([128, S], bf16, tag="ex")
                    nc.scalar.activation(ex[:], s_ps[:], AF.Exp, bias=-EXP_SHIFT)
                    nc.tensor.matmul(av_ps[:], vf[:, b, 128 * h:128 * (h + 1)],
                                     ex[:], start=(b == 0), stop=(b == N // 128 - 1))
                    nc.tensor.matmul(sum_ps[:], ones_bf[:], ex[:],
                                     start=(b == 0), stop=(b == N // 128 - 1))
                rsum = wpool.tile([1, S], f32, tag="rsum")
                nc.vector.reciprocal(rsum[:], sum_ps[:])
                bc_ps = psAcc.tile([128, S], f32, tag="acc", name="bc_ps")
                nc.tensor.matmul(bc_ps[:], ones_row[:], rsum[:], start=True,
                                 stop=True)
                bc_sb = wpool.tile([128, S], f32, tag="bc_sb")
                nc.scalar.activation(bc_sb[:], bc_ps[:], AF.Copy)
                nc.vector.tensor_tensor(oT[:, h, :], av_ps[:], bc_sb[:],
                                        op=ALU.mult)

            # -------- M = attn @ out_w.T ; H2 = h1 + M + out_b ---------------
            h2 = apool.tile([128, S // 128, H], bf16, tag="h2")
            for m in range(S // 128):
                ps = psAcc.tile([128, H], f32, tag="acc", name="m_ps")
                for c in range(HEADS):
                    nc.tensor.matmul(ps[:], oT[:, c, 128 * m:128 * (m + 1)],
                                     outwT[:, c, :], start=(c == 0),
                                     stop=(c == HEADS - 1))
                t = wpool.tile([128, H], f32, tag="ep2")
                nc.vector.tensor_tensor(t[:], ps[:], h1f[:, m, :], op=ALU.add)
                nc.vector.tensor_tensor(h2[:, m, :], t[:], outbbc[:], op=ALU.add)

            # -------- T2, P2c, AllGather ------------------------------------
            t2 = apool.tile([128, H // 128, S], bf16, tag="t2")
            for c in range(H // 128):
                tp = psAcc.tile([128, S], bf16, tag="acc", name="tp_ps")
                for m in range(S // 128):
                    nc.tensor.transpose(tp[:, 128 * m:128 * (m + 1)],
                                        h2[:, m, 128 * c:128 * (c + 1)], ident[:])
                nc.scalar.activation(t2[:, c, :], tp[:], AF.Copy)
            ag2_in = dram.tile([S, H], bf16)
            ag2_out = dram.tile([N, H], bf16)
            for m in range(S // 128):
                ps = psAcc.tile([128, H], f32, tag="acc", name="p2_ps")
                for c in range(H // 128):
                    nc.tensor.matmul(ps[:], t2[:, c, 128 * m:128 * (m + 1)],
                                     w2[:, c, :], start=(c == 0),
                                     stop=(c == H // 128 - 1))
                p2c = wpool.tile([128, H], bf16, tag="p2c")
                nc.scalar.activation(p2c[:], ps[:], AF.Identity,
                                     scale=dinv_col[:, m:m + 1])
                nc.sync.dma_start(
                    ag2_in[:].rearrange("(mm p) h -> p mm h", p=128)[:, m, :],
                    p2c[:])
            nc.gpsimd.collective_compute(
                "AllGather", ALU.bypass, ins=[ag2_in.opt()],
                outs=[ag2_out.opt()], replica_groups=[list(range(NCORES))])

            # -------- conv2 + classifier ------------------------------------
            h3 = apool.tile([128, S // 128, H], bf16, tag="h3")
            h3ps = [psAgg.tile([128, H], f32, tag="agg", name=f"h3ps{m}")
                    for m in range(4)]
            for c in range(N // 128):
                rhs = spool.tile([128, H], bf16, tag="agc2")
                nc.sync.dma_start(
                    rhs[:], ag2_out[:].rearrange("(cc p) h -> p cc h", p=128)[:, c, :])
                for m in range(S // 128):
                    nc.tensor.matmul(h3ps[m][:], acT[:, c, 128 * m:128 * (m + 1)],
                                     rhs[:], start=(c == 0), stop=(c == N // 128 - 1))
            for m in range(S // 128):
                t = wpool.tile([128, H], f32, tag="ep3")
                nc.scalar.activation(t[:], h3ps[m][:], AF.Identity,
                                     scale=dinv_col[:, m:m + 1])
                nc.vector.tensor_tensor(t[:], t[:], b2bc[:], op=ALU.add)
                nc.vector.tensor_scalar_max(h3[:, m, :], t[:], 0.0)

            t3 = apool.tile([128, H // 128, S], bf16, tag="t3")
            for c in range(H // 128):
                tp = psAcc.tile([128, S], bf16, tag="acc", name="tp_ps")
                for m in range(S // 128):
                    nc.tensor.transpose(tp[:, 128 * m:128 * (m + 1)],
                                        h3[:, m, 128 * c:128 * (c + 1)], ident[:])
                nc.scalar.activation(t3[:, c, :], tp[:], AF.Copy)

            for m in range(S // 128):
                ps = psAcc.tile([128, C], f32, tag="acc", name="oc_ps")
                for c in range(H // 128):
                    nc.tensor.matmul(ps[:], t3[:, c, 128 * m:128 * (m + 1)],
                                     wc[:, c, :], start=(c == 0),
                                     stop=(c == H // 128 - 1))
                ot = wpool.tile([128, C], f32, tag="ot")
                nc.vector.tensor_tensor(ot[:], ps[:], bccbc[:], op=ALU.add)
                nc.sync.dma_start(
                    out_t.ap().rearrange("(mm p) c -> p mm c", p=128)[:, m, :],
                    ot[:])

    nc.compile()
    return nc


def _get_compiled():
    global _compiled
    if _compiled is None:
        _compiled = _build()
    return _compiled


def _prep_inputs(x, edge_index, W1, b1, in_w, in_b, out_w, out_b, W2, b2, Wc, bc):
    x = np.asarray(x, np.float32)
    ei = np.asarray(edge_index, np.int64)
    src, dst = ei[0], ei[1]

    shared = {
        "w1": np.asarray(W1, np.float32).astype(BF16),
        "inwT": np.ascontiguousarray(np.asarray(in_w, np.float32).T).astype(BF16),
        "outwT": np.ascontiguousarray(np.asarray(out_w, np.float32).T).astype(BF16),
        "w2": np.asarray(W2, np.float32).astype(BF16),
        "wc": np.asarray(Wc, np.float32).astype(BF16),
        "b1bc": np.tile(np.asarray(b1, np.float32)[None, :], (128, 1)),
        "b2bc": np.tile(np.asarray(b2, np.float32)[None, :], (128, 1)),
        "outbbc": np.tile(np.asarray(out_b, np.float32)[None, :], (128, 1)),
        "inbvbc": np.tile(np.asarray(in_b, np.float32)[2 * H:3 * H][None, :],
                          (128, 1)),
        "bccbc": np.tile(np.asarray(bc, np.float32)[None, :], (128, 1)),
        "inbq": np.ascontiguousarray(
            np.asarray(in_b, np.float32)[0:H].reshape(HEADS, 128).T),
        "inbk": np.ascontiguousarray(
            np.asarray(in_b, np.float32)[H:2 * H].reshape(HEADS, 128).T),
        "ident": np.eye(128, dtype=np.float32).astype(BF16),
    }

    in_maps = []
    for k in range(NCORES):
        lo, hi = S * k, S * (k + 1)
        sel = (dst >= lo) & (dst < hi)
        ac = np.zeros((N, S), np.float32)
        np.add.at(ac, (src[sel], dst[sel] - lo), 1.0)
        ac[np.arange(lo, hi), np.arange(S)] += 1.0  # self loops
        m = dict(shared)
        m["acT"] = ac.astype(BF16)
        m["xT"] = np.ascontiguousarray(x[lo:hi].T).astype(BF16)
        in_maps.append(m)
    return in_maps


def kernel(**inputs) -> np.ndarray:
    from concourse import bass_utils
    nc = _get_compiled()
    in_maps = _prep_inputs(**inputs)
    res = bass_utils.run_bass_kernel_spmd(nc, in_maps,
                                          core_ids=list(range(NCORES)))
    return np.concatenate([res.results[k]["y"] for k in range(NCORES)], axis=0)


# revision 11
# speedup vs baseline: 1.0302x; 1.0302x over previous
"""EnhancedGCNWithAttention on 8 Trainium2 NeuronCores (Bass/Tile SPMD).

Strategy (node-sharded, 512 nodes per core):
  - GCN aggregation is cast as dense matmuls against a per-core count matrix
    acT[4096 src, 512 local dst] (bf16, exact small integers) built on the
    host from the edge multiset (+ self loops). Degrees, rsqrt normalization,
    and everything else arithmetic run on device.
  - A_norm @ P == diag(dinv_dst) . (acT.T @ (diag(dinv_src) . P)); the source
    scaling is a per-partition ACT scale applied to P tiles, the dst scaling
    rides the aggregation epilogue.
  - x @ W1 is computed for ALL nodes on every core (cheap), so the only
    pre-conv1 communication is an AllGather of the 2KB degree vector.
  - Attention is sharded by query rows. K^T/V are AllGathered per head in 4
    small collectives so head h's compute hides head h+1's gather. Scores
    are computed transposed (keys on partitions); exp row-sums use pairwise
    DVE adds + a ones-vector matmul; 1/sum is applied after attn@V.
"""
import sys

sys.path.insert(0, '/opt/trn_rl_repo')

import numpy as np
import ml_dtypes

N, F_IN, H, HEADS, C, E = 4096, 768, 512, 4, 32, 131072
DH = H // HEADS            # 128
NCORES = 8
S = N // NCORES            # 512 rows per core
SCALE = 1.0 / np.sqrt(DH)

BF16 = ml_dtypes.bfloat16

_compiled = None


def _build():
    from concourse import bacc, tile, mybir

    f32 = mybir.dt.float32
    bf16 = mybir.dt.bfloat16
    AF = mybir.ActivationFunctionType
    ALU = mybir.AluOpType

    NT = N // 128     # 32 node tiles
    MT = S // 128     # 4 local row tiles

    nc = bacc.Bacc("TRN2", target_bir_lowering=False, debug=False,
                   enable_asserts=False, num_devices=NCORES)

    # ---------------- I/O ----------------
    acT_in = nc.dram_tensor("acT", [N, S], bf16, kind="ExternalInput")
    xT_in = nc.dram_tensor("xT", [F_IN, N], bf16, kind="ExternalInput")
    w1_in = nc.dram_tensor("w1", [F_IN, H], bf16, kind="ExternalInput")
    inwT_in = nc.dram_tensor("inwT", [H, 3 * H], bf16, kind="ExternalInput")
    outwT_in = nc.dram_tensor("outwT", [H, H], bf16, kind="ExternalInput")
    w2_in = nc.dram_tensor("w2", [H, H], bf16, kind="ExternalInput")
    wc_in = nc.dram_tensor("wc", [H, C], bf16, kind="ExternalInput")
    b1bc_in = nc.dram_tensor("b1bc", [128, H], f32, kind="ExternalInput")
    b2bc_in = nc.dram_tensor("b2bc", [128, H], f32, kind="ExternalInput")
    outbbc_in = nc.dram_tensor("outbbc", [128, H], f32, kind="ExternalInput")
    inbvbc_in = nc.dram_tensor("inbvbc", [128, H], f32, kind="ExternalInput")
    bccbc_in = nc.dram_tensor("bccbc", [128, C], f32, kind="ExternalInput")
    inbq_in = nc.dram_tensor("inbq", [128, HEADS], f32, kind="ExternalInput")
    inbk_in = nc.dram_tensor("inbk", [128, HEADS], f32, kind="ExternalInput")
    ident_in = nc.dram_tensor("ident", [128, 128], bf16, kind="ExternalInput")
    out_t = nc.dram_tensor("y", [S, C], f32, kind="ExternalOutput")

    with tile.TileContext(nc) as tc:
        with tc.tile_pool(name="const", bufs=1) as cpool, \
             tc.tile_pool(name="acts", bufs=1) as apool, \
             tc.tile_pool(name="stream", bufs=3) as spool, \
             tc.tile_pool(name="kv", bufs=2) as kvpool, \
             tc.tile_pool(name="work", bufs=2) as wpool, \
             tc.tile_pool(name="psAgg", bufs=2, space="PSUM") as psAgg, \
             tc.tile_pool(name="psSc", bufs=2, space="PSUM") as psSc, \
             tc.tile_pool(name="psAv", bufs=1, space="PSUM") as psAv, \
             tc.tile_pool(name="psSum", bufs=1, space="PSUM") as psSum, \
             tc.tile_pool(name="dram", bufs=1, space="DRAM") as dram:

            # -------- constant loads --------
            def ld(name, t_in, shape, dt, pool=cpool, view=None):
                t = pool.tile(shape, dt, tag=name, name=name + "_sb")
                src = t_in.ap() if view is None else view
                nc.sync.dma_start(t[:], src)
                return t

            # acT in 4 pieces so the degree matmuls can pipeline behind it
            acT = cpool.tile([128, NT, S], bf16, tag="acT", name="acT_sb")
            acT_v = acT_in.ap().rearrange("(c p) s -> p c s", p=128)
            for g in range(4):
                nc.sync.dma_start(acT[:, 8 * g:8 * (g + 1), :],
                                  acT_v[:, 8 * g:8 * (g + 1), :])
            w1 = ld("w1", w1_in, [128, F_IN // 128, H], bf16,
                    view=w1_in.ap().rearrange("(c p) h -> p c h", p=128))
            inwT = ld("inwT", inwT_in, [128, H // 128, 3 * H], bf16,
                      view=inwT_in.ap().rearrange("(c p) h -> p c h", p=128))
            outwT = ld("outwT", outwT_in, [128, H // 128, H], bf16,
                       view=outwT_in.ap().rearrange("(c p) h -> p c h", p=128))
            w2 = ld("w2", w2_in, [128, H // 128, H], bf16,
                    view=w2_in.ap().rearrange("(c p) h -> p c h", p=128))
            wc = ld("wc", wc_in, [128, H // 128, C], bf16,
                    view=wc_in.ap().rearrange("(c p) h -> p c h", p=128))
            b1bc = ld("b1bc", b1bc_in, [128, H], f32)
            b2bc = ld("b2bc", b2bc_in, [128, H], f32)
            outbbc = ld("outbbc", outbbc_in, [128, H], f32)
            inbvbc = ld("inbvbc", inbvbc_in, [128, H], f32)
            bccbc = ld("bccbc", bccbc_in, [128, C], f32)
            inbq = ld("inbq", inbq_in, [128, HEADS], f32)
            inbk = ld("inbk", inbk_in, [128, HEADS], f32)
            ident = ld("ident", ident_in, [128, 128], bf16)

            ones_bf = cpool.tile([128, 1], bf16, tag="ones_bf", name="ones_bf")
            nc.vector.memset(ones_bf[:], 1.0)
            ones_row = cpool.tile([1, 128], f32, tag="ones_row", name="ones_row")
            nc.vector.memset(ones_row[:], 1.0)
            one_11 = cpool.tile([1, 1], f32, tag="one_11", name="one_11")
            nc.vector.memset(one_11[:], 1.0)

            # -------- degrees + dinv (own shard), AllGather dinv ------------
            deg_ps = psSum.tile([1, S], f32, tag="sum", name="deg_ps")
            for c in range(NT):
                nc.tensor.matmul(deg_ps[:], ones_bf[:], acT[:, c, :],
                                 start=(c == 0), stop=(c == NT - 1))
            rdeg = cpool.tile([1, S], f32, tag="rdeg", name="rdeg")
            nc.vector.reciprocal(rdeg[:], deg_ps[:])
            dinv_row = cpool.tile([1, S], f32, tag="dinv_row", name="dinv_row")
            nc.scalar.activation(dinv_row[:], rdeg[:], AF.Sqrt)
            # own column layout for the aggregation epilogues
            dinv_col = cpool.tile([128, MT], f32, tag="dinv_col", name="dinv_col")
            for m in range(MT):
                dc_ps = psAv.tile([128, 1], f32, tag="av", name="dc_ps")
                nc.tensor.matmul(dc_ps[:], dinv_row[:, 128 * m:128 * (m + 1)],
                                 one_11[:], start=True, stop=True)
                nc.scalar.activation(dinv_col[:, m:m + 1], dc_ps[:], AF.Copy)
            agd_in = dram.tile([1, S], f32)
            agd_out = dram.tile([NCORES, S], f32, addr_space="Shared")
            nc.sync.dma_start(agd_in[:], dinv_row[:])
            nc.gpsimd.collective_compute(
                "AllGather", ALU.bypass, ins=[agd_in.opt()],
                outs=[agd_out.opt()], replica_groups=[list(range(NCORES))])
            # dinv_colf[p, 4c + t] = dinv_full[512c + 128t + p]
            dinv_colf = cpool.tile([128, NT], f32, tag="dinv_colf",
                                   name="dinv_colf")
            nc.sync.dma_start(dinv_colf[:],
                              agd_out[:].rearrange("c (t p) -> p (c t)", p=128))

            # -------- p1c[t] = dinv .* (x @ W1) for ALL nodes ---------------
            p1c = apool.tile([128, NT, H], bf16, tag="p1c", name="p1c_sb")
            xT_v = xT_in.ap().rearrange("(c p) (t q) -> p t c q", p=128, q=128)
            for t in range(NT):
                xt_t = spool.tile([128, F_IN // 128, 128], bf16, tag="xt",
                                  name="xt_t")
                nc.sync.dma_start(xt_t[:], xT_v[:, t, :, :])
                ps = psAgg.tile([128, H], f32, tag="agg", name="p1_ps")
                for c in range(F_IN // 128):
                    nc.tensor.matmul(ps[:], xt_t[:, c, :], w1[:, c, :],
                                     start=(c == 0), stop=(c == F_IN // 128 - 1))
                nc.scalar.activation(p1c[:, t, :], ps[:], AF.Identity,
                                     scale=dinv_colf[:, t:t + 1])

            # -------- conv1: h1 = relu(dinv .* (acT.T @ p1c) + b1) ----------
            h1f = apool.tile([128, MT, H], f32, tag="h1f", name="h1f_sb")
            for half in range(2):
                hps = [psAgg.tile([128, H], f32, tag="agg", name=f"h1ps{half}{i}")
                       for i in range(2)]
                for c in range(NT):
                    for i in range(2):
                        m = 2 * half + i
                        nc.tensor.matmul(hps[i][:],
                                         acT[:, c, 128 * m:128 * (m + 1)],
                                         p1c[:, c, :], start=(c == 0),
                                         stop=(c == NT - 1))
                for i in range(2):
                    m = 2 * half + i
                    t = wpool.tile([128, H], f32, tag="ep1", name="ep1_t")
                    nc.scalar.activation(t[:], hps[i][:], AF.Identity,
                                         scale=dinv_col[:, m:m + 1])
                    nc.vector.tensor_tensor(t[:], t[:], b1bc[:], op=ALU.add)
                    nc.vector.tensor_scalar_max(h1f[:, m, :], t[:], 0.0)

            # -------- T1 = h1^T --------
            t1 = apool.tile([128, H // 128, S], bf16, tag="t1", name="t1_sb")
            identf = cpool.tile([128, 128], f32, tag="identf", name="identf")
            nc.vector.tensor_copy(identf[:], ident[:])
            for c in range(H // 128):
                tp = psSc.tile([128, S], f32, tag="sc", name="t1_ps")
                for m in range(MT):
                    nc.tensor.transpose(tp[:, 128 * m:128 * (m + 1)],
                                        h1f[:, m, 128 * c:128 * (c + 1)],
                                        identf[:])
                nc.vector.tensor_copy(t1[:, c, :], tp[:])

            # -------- qkv; V first, then per-head K^T + AllGather -----------
            agkv_in = [dram.tile([2 * DH, S], bf16, name=f"agkvi{h}")
                       for h in range(HEADS)]
            agkv_out = [dram.tile([NCORES * 2 * DH, S], bf16, name=f"agkvo{h}",
                                  addr_space="Shared")
                        for h in range(HEADS)]
            vv = apool.tile([128, MT, H], bf16, tag="vvqt", name="vv_sb")
            for m in range(MT):
                ps = psAgg.tile([128, H], f32, tag="agg", name="v_ps")
                for c in range(H // 128):
                    nc.tensor.matmul(ps[:], t1[:, c, 128 * m:128 * (m + 1)],
                                     inwT[:, c, 2 * H:3 * H], start=(c == 0),
                                     stop=(c == H // 128 - 1))
                nc.vector.tensor_tensor(vv[:, m, :], ps[:], inbvbc[:], op=ALU.add)
            kt = apool.tile([128, HEADS, S], bf16, tag="kt", name="kt_sb")
            for h in range(HEADS):
                ps2 = psAgg.tile([128, S], f32, tag="agg", name="k_ps")
                for c in range(H // 128):
                    nc.tensor.matmul(ps2[:],
                                     inwT[:, c, H + 128 * h:H + 128 * (h + 1)],
                                     t1[:, c, :], start=(c == 0),
                                     stop=(c == H // 128 - 1))
                nc.vector.tensor_scalar_add(kt[:, h, :], ps2[:], inbk[:, h:h + 1])
                nc.sync.dma_start(agkv_in[h][0:DH, :], kt[:, h, :])
                nc.sync.dma_start(
                    agkv_in[h][DH:2 * DH, :].rearrange("p (m v) -> p m v", m=MT),
                    vv[:, :, 128 * h:128 * (h + 1)])
                nc.gpsimd.collective_compute(
                    "AllGather", ALU.bypass, ins=[agkv_in[h].opt()],
                    outs=[agkv_out[h].opt()],
                    replica_groups=[list(range(NCORES))])

            qt = apool.tile([128, HEADS, S], bf16, tag="vvqt", name="qt_sb")
            inbq_s = cpool.tile([128, HEADS], f32, tag="inbq_s", name="inbq_s")
            nc.vector.tensor_scalar_mul(inbq_s[:], inbq[:], SCALE)
            for h in range(HEADS):
                ps = psAgg.tile([128, S], f32, tag="agg", name="q_ps")
                for c in range(H // 128):
                    nc.tensor.matmul(ps[:], inwT[:, c, 128 * h:128 * (h + 1)],
                                     t1[:, c, :], start=(c == 0),
                                     stop=(c == H // 128 - 1))
                nc.vector.tensor_scalar(qt[:, h, :], ps[:], SCALE,
                                        inbq_s[:, h:h + 1], op0=ALU.mult,
                                        op1=ALU.add)

            # -------- attention, heads pipelined over their AllGathers ------
            oT = apool.tile([128, HEADS, S], bf16, tag="oT", name="oT_sb")
            for h in range(HEADS):
                agv = agkv_out[h][:].rearrange("(c z p) s -> p z c s", p=128, z=2)
                kth = kvpool.tile([128, NCORES, S], bf16, tag="kth", name="kth")
                nc.sync.dma_start(kth[:], agv[:, 0, :, :])
                vh = kvpool.tile([128, NT, DH], bf16, tag="vh", name="vh")
                nc.sync.dma_start(
                    vh[:].rearrange("p (c t) v -> p c t v", c=NCORES),
                    agv[:, 1, :, :].rearrange("p c (t v) -> p c t v", t=MT))
                kthf = kth[:].rearrange("p c s -> p (c s)")
                av_ps = psAv.tile([128, S], f32, tag="av", name="av_ps")
                sum_ps = psSum.tile([1, S], f32, tag="sum", name="sum_ps")
                for b in range(NT):
                    s_ps = psSc.tile([128, S], f32, tag="sc", name="s_ps")
                    nc.tensor.matmul(s_ps[:], kthf[:, 128 * b:128 * (b + 1)],
                                     qt[:, h, :], start=True, stop=True)
                    ex = wpool.tile([128, S], bf16, tag="ex", name="ex_t")
                    nc.scalar.activation(ex[:], s_ps[:], AF.Exp)
                    nc.tensor.matmul(av_ps[:], vh[:, b, :], ex[:],
                                     start=(b == 0), stop=(b == NT - 1))
                    nc.tensor.matmul(sum_ps[:], ones_bf[:], ex[:],
                                     start=(b == 0), stop=(b == NT - 1))
                rsum = wpool.tile([1, S], f32, tag="rsum", name="rsum_t", bufs=1)
                nc.vector.reciprocal(rsum[:], sum_ps[:])
                bc_ps = psSc.tile([128, S], f32, tag="sc", name="bc_ps")
                nc.tensor.matmul(bc_ps[:], ones_row[:], rsum[:], start=True,
                                 stop=True)
                bc_sb = wpool.tile([128, S], f32, tag="bc_sb", name="bc_sb", bufs=1)
                nc.scalar.activation(bc_sb[:], bc_ps[:], AF.Copy)
                nc.vector.tensor_tensor(oT[:, h, :], av_ps[:], bc_sb[:],
                                        op=ALU.mult)

            # -------- M = attn @ out_w.T ; H2 = h1 + M + out_b --------------
            h2 = apool.tile([128, MT, H], bf16, tag="h23", name="h2_sb")
            for m in range(MT):
                ps = psAgg.tile([128, H], f32, tag="agg", name="m_ps")
                for c in range(HEADS):
                    nc.tensor.matmul(ps[:], oT[:, c, 128 * m:128 * (m + 1)],
                                     outwT[:, c, :], start=(c == 0),
                                     stop=(c == HEADS - 1))
                t = wpool.tile([128, H], f32, tag="ep2", name="ep2_t")
                nc.vector.tensor_tensor(t[:], ps[:], h1f[:, m, :], op=ALU.add)
                nc.vector.tensor_tensor(h2[:, m, :], t[:], outbbc[:], op=ALU.add)

            # -------- T2, P2c, AllGather (2 halves) -------------------------
            t2 = apool.tile([128, H // 128, S], bf16, tag="t23", name="t2_sb")
            for c in range(H // 128):
                tp = psSc.tile([128, S], bf16, tag="sc", name="t2_ps")
                for m in range(MT):
                    nc.tensor.transpose(tp[:, 128 * m:128 * (m + 1)],
                                        h2[:, m, 128 * c:128 * (c + 1)], ident[:])
                nc.vector.tensor_copy(t2[:, c, :], tp[:])
            ag2_in = [dram.tile([S // 2, H], bf16, name=f"ag2i{hf}")
                      for hf in range(2)]
            ag2_out = [dram.tile([N // 2, H], bf16, name=f"ag2o{hf}",
                              addr_space="Shared")
                       for hf in range(2)]
            for half in range(2):
                for i in range(2):
                    m = 2 * half + i
                    ps = psAgg.tile([128, H], f32, tag="agg", name="p2_ps")
                    for c in range(H // 128):
                        nc.tensor.matmul(ps[:], t2[:, c, 128 * m:128 * (m + 1)],
                                         w2[:, c, :], start=(c == 0),
                                         stop=(c == H // 128 - 1))
                    p2c = wpool.tile([128, H], bf16, tag="p2c", name="p2c_t")
                    nc.scalar.activation(p2c[:], ps[:], AF.Identity,
                                         scale=dinv_col[:, m:m + 1])
                    nc.sync.dma_start(
                        ag2_in[half][:].rearrange("(mm p) h -> p mm h",
                                                  p=128)[:, i, :],
                        p2c[:])
                nc.gpsimd.collective_compute(
                    "AllGather", ALU.bypass, ins=[ag2_in[half].opt()],
                    outs=[ag2_out[half].opt()],
                    replica_groups=[list(range(NCORES))])

            # -------- conv2 + classifier ------------------------------------
            h3 = apool.tile([128, MT, H], bf16, tag="h23", name="h3_sb")
            for hf in range(2):
                hps = [psAgg.tile([128, H], f32, tag="agg", name=f"h3ps{hf}{i}")
                       for i in range(2)]
                first, last = (0, 0, 0), (1, NCORES - 1, 1)
                for half in range(2):
                    agv2 = ag2_out[half][:].rearrange("(cc p) h -> p cc h", p=128)
                    for cc in range(NCORES):
                        rhs = spool.tile([128, 2, H], bf16, tag="agc2",
                                         name="agc2_t")
                        nc.sync.dma_start(rhs[:],
                                          agv2[:, 2 * cc:2 * (cc + 1), :])
                        for i in range(2):
                            for k in range(2):
                                m = 2 * hf + k
                                nc.tensor.matmul(
                                    hps[k][:],
                                    acT[:, 4 * cc + 2 * half + i,
                                        128 * m:128 * (m + 1)],
                                    rhs[:, i, :],
                                    start=((half, cc, i) == first),
                                    stop=((half, cc, i) == last))
                for k in range(2):
                    m = 2 * hf + k
                    t = wpool.tile([128, H], f32, tag="ep3", name="ep3_t")
                    nc.scalar.activation(t[:], hps[k][:], AF.Identity,
                                         scale=dinv_col[:, m:m + 1])
                    nc.vector.tensor_tensor(t[:], t[:], b2bc[:], op=ALU.add)
                    nc.vector.tensor_scalar_max(h3[:, m, :], t[:], 0.0)

            t3 = apool.tile([128, H // 128, S], bf16, tag="t23", name="t3_sb")
            for c in range(H // 128):
                tp = psSc.tile([128, S], bf16, tag="sc", name="t3_ps")
                for m in range(MT):
                    nc.tensor.transpose(tp[:, 128 * m:128 * (m + 1)],
                                        h3[:, m, 128 * c:128 * (c + 1)], ident[:])
                nc.vector.tensor_copy(t3[:, c, :], tp[:])

            for m in range(MT):
                ps = psAv.tile([128, C], f32, tag="av", name="oc_ps")
                for c in range(H // 128):
                    nc.tensor.matmul(ps[:], t3[:, c, 128 * m:128 * (m + 1)],
                                     wc[:, c, :], start=(c == 0),
                                     stop=(c == H // 128 - 1))
                ot = wpool.tile([128, C], f32, tag="ot", name="ot_t", bufs=1)
                nc.vector.tensor_tensor(ot[:], ps[:], bccbc[:], op=ALU.add)
                nc.sync.dma_start(
                    out_t.ap().rearrange("(mm p) c -> p mm c", p=128)[:, m, :],
                    ot[:])

    nc.compile()
    return nc


def _get_compiled():
    global _compiled
    if _compiled is None:
        _compiled = _build()
    return _compiled


def _prep_inputs(x, edge_index, W1, b1, in_w, in_b, out_w, out_b, W2, b2, Wc, bc):
    x = np.asarray(x, np.float32)
    ei = np.asarray(edge_index, np.int64)
    src, dst = ei[0], ei[1]

    shared = {
        "xT": np.ascontiguousarray(x.T).astype(BF16),
        "w1": np.asarray(W1, np.float32).astype(BF16),
        "inwT": np.ascontiguousarray(np.asarray(in_w, np.float32).T).astype(BF16),
        "outwT": np.ascontiguousarray(np.asarray(out_w, np.float32).T).astype(BF16),
        "w2": np.asarray(W2, np.float32).astype(BF16),
        "wc": np.asarray(Wc, np.float32).astype(BF16),
        "b1bc": np.tile(np.asarray(b1, np.float32)[None, :], (128, 1)),
        "b2bc": np.tile(np.asarray(b2, np.float32)[None, :], (128, 1)),
        "outbbc": np.tile(np.asarray(out_b, np.float32)[None, :], (128, 1)),
        "inbvbc": np.tile(np.asarray(in_b, np.float32)[2 * H:3 * H][None, :],
                          (128, 1)),
        "bccbc": np.tile(np.asarray(bc, np.float32)[None, :], (128, 1)),
        "inbq": np.ascontiguousarray(
            np.asarray(in_b, np.float32)[0:H].reshape(HEADS, 128).T),
        "inbk": np.ascontiguousarray(
            np.asarray(in_b, np.float32)[H:2 * H].reshape(HEADS, 128).T),
        "ident": np.eye(128, dtype=np.float32).astype(BF16),
    }

    in_maps = []
    for k in range(NCORES):
        lo, hi = S * k, S * (k + 1)
        sel = (dst >= lo) & (dst < hi)
        ac = np.zeros((N, S), np.float32)
        np.add.at(ac, (src[sel], dst[sel] - lo), 1.0)
        ac[np.arange(lo, hi), np.arange(S)] += 1.0  # self loops
        m = dict(shared)
        m["acT"] = ac.astype(BF16)
        in_maps.append(m)
    return in_maps


def kernel(**inputs) -> np.ndarray:
    from concourse import bass_utils
    nc = _get_compiled()
    in_maps = _prep_inputs(**inputs)
    res = bass_utils.run_bass_kernel_spmd(nc, in_maps,
                                          core_ids=list(range(NCORES)))
    return np.concatenate([res.results[k]["y"] for k in range(NCORES)], axis=0)


# revision 12
# speedup vs baseline: 1.1166x; 1.0840x over previous
"""EnhancedGCNWithAttention on 8 Trainium2 NeuronCores (Bass/Tile SPMD).

Strategy (node-sharded, 512 nodes per core):
  - GCN aggregation is cast as dense matmuls against a per-core count matrix
    acT[4096 src, 512 local dst] (bf16, exact small integers) built on the
    host from the edge multiset (+ self loops). Degrees, rsqrt normalization,
    and everything else arithmetic run on device.
  - A_norm @ P == diag(dinv_dst) . (acT.T @ (diag(dinv_src) . P)); the source
    scaling is a per-partition ACT scale applied to P tiles, the dst scaling
    rides the aggregation epilogue.
  - x @ W1 is computed for ALL nodes on every core (cheap), so the only
    pre-conv1 communication is an AllGather of the 2KB degree vector.
  - Attention is sharded by query rows. K^T/V are AllGathered per head in 4
    small collectives so head h's compute hides head h+1's gather. Scores
    are computed transposed (keys on partitions); exp row-sums use pairwise
    DVE adds + a ones-vector matmul; 1/sum is applied after attn@V.
"""
import sys

sys.path.insert(0, '/opt/trn_rl_repo')

import numpy as np
import ml_dtypes

N, F_IN, H, HEADS, C, E = 4096, 768, 512, 4, 32, 131072
DH = H // HEADS            # 128
NCORES = 8
S = N // NCORES            # 512 rows per core
SCALE = 1.0 / np.sqrt(DH)

BF16 = ml_dtypes.bfloat16

_compiled = None


def _build():
    from concourse import bacc, tile, mybir

    f32 = mybir.dt.float32
    bf16 = mybir.dt.bfloat16
    AF = mybir.ActivationFunctionType
    ALU = mybir.AluOpType

    NT = N // 128     # 32 node tiles
    MT = S // 128     # 4 local row tiles

    nc = bacc.Bacc("TRN2", target_bir_lowering=False, debug=False,
                   enable_asserts=False, num_devices=NCORES)

    # ---------------- I/O ----------------
    acT_in = nc.dram_tensor("acT", [N, S], bf16, kind="ExternalInput")
    xT_in = nc.dram_tensor("xT", [F_IN, N], bf16, kind="ExternalInput")
    w1_in = nc.dram_tensor("w1", [F_IN, H], bf16, kind="ExternalInput")
    inwT_in = nc.dram_tensor("inwT", [H, 3 * H], bf16, kind="ExternalInput")
    outwT_in = nc.dram_tensor("outwT", [H, H], bf16, kind="ExternalInput")
    w2_in = nc.dram_tensor("w2", [H, H], bf16, kind="ExternalInput")
    wc_in = nc.dram_tensor("wc", [H, C], bf16, kind="ExternalInput")
    b1bc_in = nc.dram_tensor("b1bc", [128, H], f32, kind="ExternalInput")
    b2bc_in = nc.dram_tensor("b2bc", [128, H], f32, kind="ExternalInput")
    outbbc_in = nc.dram_tensor("outbbc", [128, H], f32, kind="ExternalInput")
    inbvbc_in = nc.dram_tensor("inbvbc", [128, H], f32, kind="ExternalInput")
    bccbc_in = nc.dram_tensor("bccbc", [128, C], f32, kind="ExternalInput")
    inbq_in = nc.dram_tensor("inbq", [128, HEADS], f32, kind="ExternalInput")
    inbk_in = nc.dram_tensor("inbk", [128, HEADS], f32, kind="ExternalInput")
    ident_in = nc.dram_tensor("ident", [128, 128], bf16, kind="ExternalInput")
    out_t = nc.dram_tensor("y", [S, C], f32, kind="ExternalOutput")

    with tile.TileContext(nc) as tc:
        with tc.tile_pool(name="const", bufs=1) as cpool, \
             tc.tile_pool(name="acts", bufs=1) as apool, \
             tc.tile_pool(name="stream", bufs=3) as spool, \
             tc.tile_pool(name="kv", bufs=2) as kvpool, \
             tc.tile_pool(name="work", bufs=2) as wpool, \
             tc.tile_pool(name="psAgg", bufs=2, space="PSUM") as psAgg, \
             tc.tile_pool(name="psSc", bufs=2, space="PSUM") as psSc, \
             tc.tile_pool(name="psAv", bufs=1, space="PSUM") as psAv, \
             tc.tile_pool(name="psSum", bufs=1, space="PSUM") as psSum, \
             tc.tile_pool(name="dram", bufs=1, space="DRAM") as dram:

            # -------- constant loads --------
            def ld(name, t_in, shape, dt, pool=cpool, view=None):
                t = pool.tile(shape, dt, tag=name, name=name + "_sb")
                src = t_in.ap() if view is None else view
                nc.sync.dma_start(t[:], src)
                return t

            # acT in 4 pieces so the degree matmuls can pipeline behind it
            acT = cpool.tile([128, NT, S], bf16, tag="acT", name="acT_sb")
            acT_v = acT_in.ap().rearrange("(c p) s -> p c s", p=128)
            for g in range(4):
                nc.sync.dma_start(acT[:, 8 * g:8 * (g + 1), :],
                                  acT_v[:, 8 * g:8 * (g + 1), :])
            w1 = ld("w1", w1_in, [128, F_IN // 128, H], bf16,
                    view=w1_in.ap().rearrange("(c p) h -> p c h", p=128))
            inwT = ld("inwT", inwT_in, [128, H // 128, 3 * H], bf16,
                      view=inwT_in.ap().rearrange("(c p) h -> p c h", p=128))
            outwT = ld("outwT", outwT_in, [128, H // 128, H], bf16,
                       view=outwT_in.ap().rearrange("(c p) h -> p c h", p=128))
            w2 = ld("w2", w2_in, [128, H // 128, H], bf16,
                    view=w2_in.ap().rearrange("(c p) h -> p c h", p=128))
            wc = ld("wc", wc_in, [128, H // 128, C], bf16,
                    view=wc_in.ap().rearrange("(c p) h -> p c h", p=128))
            b1bc = ld("b1bc", b1bc_in, [128, H], f32)
            b2bc = ld("b2bc", b2bc_in, [128, H], f32)
            outbbc = ld("outbbc", outbbc_in, [128, H], f32)
            inbvbc = ld("inbvbc", inbvbc_in, [128, H], f32)
            bccbc = ld("bccbc", bccbc_in, [128, C], f32)
            inbq = ld("inbq", inbq_in, [128, HEADS], f32)
            inbk = ld("inbk", inbk_in, [128, HEADS], f32)
            ident = ld("ident", ident_in, [128, 128], bf16)

            ones_bf = cpool.tile([128, 1], bf16, tag="ones_bf", name="ones_bf")
            nc.vector.memset(ones_bf[:], 1.0)
            ones_row = cpool.tile([1, 128], f32, tag="ones_row", name="ones_row")
            nc.vector.memset(ones_row[:], 1.0)
            one_11 = cpool.tile([1, 1], f32, tag="one_11", name="one_11")
            nc.vector.memset(one_11[:], 1.0)

            # -------- degrees + dinv (own shard), AllGather dinv ------------
            deg_ps = psSum.tile([1, S], f32, tag="sum", name="deg_ps")
            for c in range(NT):
                nc.tensor.matmul(deg_ps[:], ones_bf[:], acT[:, c, :],
                                 start=(c == 0), stop=(c == NT - 1))
            rdeg = cpool.tile([1, S], f32, tag="rdeg", name="rdeg")
            nc.vector.reciprocal(rdeg[:], deg_ps[:])
            dinv_row = cpool.tile([1, S], f32, tag="dinv_row", name="dinv_row")
            nc.scalar.activation(dinv_row[:], rdeg[:], AF.Sqrt)
            # own column layout for the aggregation epilogues
            dinv_col = cpool.tile([128, MT], f32, tag="dinv_col", name="dinv_col")
            for m in range(MT):
                dc_ps = psAv.tile([128, 1], f32, tag="av", name="dc_ps")
                nc.tensor.matmul(dc_ps[:], dinv_row[:, 128 * m:128 * (m + 1)],
                                 one_11[:], start=True, stop=True)
                nc.scalar.activation(dinv_col[:, m:m + 1], dc_ps[:], AF.Copy)
            agd_in = dram.tile([1, S], f32)
            agd_out = dram.tile([NCORES, S], f32, addr_space="Shared")
            nc.sync.dma_start(agd_in[:], dinv_row[:])
            nc.gpsimd.collective_compute(
                "AllGather", ALU.bypass, ins=[agd_in.opt()],
                outs=[agd_out.opt()], replica_groups=[list(range(NCORES))])
            # dinv_colf[p, 4c + t] = dinv_full[512c + 128t + p]
            dinv_colf = cpool.tile([128, NT], f32, tag="dinv_colf",
                                   name="dinv_colf")
            nc.sync.dma_start(dinv_colf[:],
                              agd_out[:].rearrange("c (t p) -> p (c t)", p=128))

            # -------- p1c[t] = dinv .* (x @ W1) for ALL nodes ---------------
            p1c = apool.tile([128, NT, H], bf16, tag="p1c", name="p1c_sb")
            xT_v = xT_in.ap().rearrange("(c p) (t q) -> p t c q", p=128, q=128)
            for t in range(NT):
                xt_t = spool.tile([128, F_IN // 128, 128], bf16, tag="xt",
                                  name="xt_t")
                nc.sync.dma_start(xt_t[:], xT_v[:, t, :, :])
                ps = psAgg.tile([128, H], f32, tag="agg", name="p1_ps")
                for c in range(F_IN // 128):
                    nc.tensor.matmul(ps[:], xt_t[:, c, :], w1[:, c, :],
                                     start=(c == 0), stop=(c == F_IN // 128 - 1))
                nc.scalar.activation(p1c[:, t, :], ps[:], AF.Identity,
                                     scale=dinv_colf[:, t:t + 1])

            # -------- conv1: h1 = relu(dinv .* (acT.T @ p1c) + b1) ----------
            h1f = apool.tile([128, MT, H], f32, tag="h1f", name="h1f_sb")
            for half in range(2):
                hps = [psAgg.tile([128, H], f32, tag="agg", name=f"h1ps{half}{i}")
                       for i in range(2)]
                for c in range(NT):
                    for i in range(2):
                        m = 2 * half + i
                        nc.tensor.matmul(hps[i][:],
                                         acT[:, c, 128 * m:128 * (m + 1)],
                                         p1c[:, c, :], start=(c == 0),
                                         stop=(c == NT - 1))
                for i in range(2):
                    m = 2 * half + i
                    t = wpool.tile([128, H], f32, tag="ep1", name="ep1_t")
                    nc.scalar.activation(t[:], hps[i][:], AF.Identity,
                                         scale=dinv_col[:, m:m + 1])
                    nc.vector.tensor_tensor(t[:], t[:], b1bc[:], op=ALU.add)
                    nc.vector.tensor_scalar_max(h1f[:, m, :], t[:], 0.0)

            # -------- T1 = h1^T --------
            t1 = apool.tile([128, H // 128, S], bf16, tag="t1", name="t1_sb")
            identf = cpool.tile([128, 128], f32, tag="identf", name="identf")
            nc.vector.tensor_copy(identf[:], ident[:])
            for c in range(H // 128):
                tp = psSc.tile([128, S], f32, tag="sc", name="t1_ps")
                for m in range(MT):
                    nc.tensor.transpose(tp[:, 128 * m:128 * (m + 1)],
                                        h1f[:, m, 128 * c:128 * (c + 1)],
                                        identf[:])
                nc.vector.tensor_copy(t1[:, c, :], tp[:])

            # -------- qkv; V first, then per-head K^T + AllGather -----------
            agkv_in = [dram.tile([2 * DH, S], bf16, name=f"agkvi{h}")
                       for h in range(HEADS)]
            agkv_out = [dram.tile([NCORES * 2 * DH, S], bf16, name=f"agkvo{h}",
                                  addr_space="Shared")
                        for h in range(HEADS)]
            vv = apool.tile([128, MT, H], bf16, tag="vvqt", name="vv_sb")
            for m in range(MT):
                ps = psAgg.tile([128, H], f32, tag="agg", name="v_ps")
                for c in range(H // 128):
                    nc.tensor.matmul(ps[:], t1[:, c, 128 * m:128 * (m + 1)],
                                     inwT[:, c, 2 * H:3 * H], start=(c == 0),
                                     stop=(c == H // 128 - 1))
                nc.vector.tensor_tensor(vv[:, m, :], ps[:], inbvbc[:], op=ALU.add)
            kt = apool.tile([128, HEADS, S], bf16, tag="kt", name="kt_sb")
            for h in range(HEADS):
                ps2 = psAgg.tile([128, S], f32, tag="agg", name="k_ps")
                for c in range(H // 128):
                    nc.tensor.matmul(ps2[:],
                                     inwT[:, c, H + 128 * h:H + 128 * (h + 1)],
                                     t1[:, c, :], start=(c == 0),
                                     stop=(c == H // 128 - 1))
                nc.vector.tensor_scalar_add(kt[:, h, :], ps2[:], inbk[:, h:h + 1])
                nc.sync.dma_start(agkv_in[h][0:DH, :], kt[:, h, :])
                nc.sync.dma_start(
                    agkv_in[h][DH:2 * DH, :].rearrange("p (m v) -> p m v", m=MT),
                    vv[:, :, 128 * h:128 * (h + 1)])
                nc.gpsimd.collective_compute(
                    "AllGather", ALU.bypass, ins=[agkv_in[h].opt()],
                    outs=[agkv_out[h].opt()],
                    replica_groups=[list(range(NCORES))])

            qt = apool.tile([128, HEADS, S], bf16, tag="vvqt", name="qt_sb")
            inbq_s = cpool.tile([128, HEADS], f32, tag="inbq_s", name="inbq_s")
            nc.vector.tensor_scalar_mul(inbq_s[:], inbq[:], SCALE)
            for h in range(HEADS):
                ps = psAgg.tile([128, S], f32, tag="agg", name="q_ps")
                for c in range(H // 128):
                    nc.tensor.matmul(ps[:], inwT[:, c, 128 * h:128 * (h + 1)],
                                     t1[:, c, :], start=(c == 0),
                                     stop=(c == H // 128 - 1))
                nc.vector.tensor_scalar(qt[:, h, :], ps[:], SCALE,
                                        inbq_s[:, h:h + 1], op0=ALU.mult,
                                        op1=ALU.add)

            # -------- attention, heads pipelined over their AllGathers ------
            oT = apool.tile([128, HEADS, S], bf16, tag="oT", name="oT_sb")
            for h in range(HEADS):
                agv = agkv_out[h][:].rearrange("(c z p) s -> p z c s", p=128, z=2)
                kth = kvpool.tile([128, NCORES, S], bf16, tag="kth", name="kth")
                nc.sync.dma_start(kth[:], agv[:, 0, :, :])
                vh = kvpool.tile([128, NT, DH], bf16, tag="vh", name="vh")
                nc.sync.dma_start(
                    vh[:].rearrange("p (c t) v -> p c t v", c=NCORES),
                    agv[:, 1, :, :].rearrange("p c (t v) -> p c t v", t=MT))
                kthf = kth[:].rearrange("p c s -> p (c s)")
                av_ps = psAv.tile([128, S], f32, tag="av", name="av_ps")
                sum_ps = psSum.tile([1, S], f32, tag="sum", name="sum_ps")
                l1_prev = None
                for g in range(NT // 2):            # 2 key blocks per group
                    sca = psSc.tile([128, 2, S], f32, tag="sc", name="sca_ps")
                    for j in range(2):
                        b = 2 * g + j
                        nc.tensor.matmul(sca[:, j, :],
                                         kthf[:, 128 * b:128 * (b + 1)],
                                         qt[:, h, :], start=True, stop=True)
                    ex = wpool.tile([128, 2, S], bf16, tag="ex", name="ex_t")
                    nc.scalar.activation(ex[:], sca[:], AF.Exp)
                    for j in range(2):
                        nc.tensor.matmul(av_ps[:], vh[:, 2 * g + j, :],
                                         ex[:, j, :],
                                         start=(g == 0 and j == 0),
                                         stop=(g == NT // 2 - 1 and j == 1))
                    l1 = wpool.tile([128, S], bf16, tag="l1", name="l1_t")
                    nc.vector.tensor_tensor(l1[:], ex[:, 0, :], ex[:, 1, :],
                                            op=ALU.add)
                    if g % 2 == 0:
                        l1_prev = l1
                    else:
                        l2 = wpool.tile([128, S], bf16, tag="l2", name="l2_t", bufs=1)
                        nc.vector.tensor_tensor(l2[:], l1_prev[:], l1[:],
                                                op=ALU.add)
                        nc.tensor.matmul(sum_ps[:], ones_bf[:], l2[:],
                                         start=(g == 1),
                                         stop=(g == NT // 2 - 1))
                rsum = wpool.tile([1, S], f32, tag="rsum", name="rsum_t", bufs=1)
                nc.vector.reciprocal(rsum[:], sum_ps[:])
                bc_ps = psSc.tile([128, S], f32, tag="sc", name="bc_ps")
                nc.tensor.matmul(bc_ps[:], ones_row[:], rsum[:], start=True,
                                 stop=True)
                bc_sb = wpool.tile([128, S], f32, tag="bc_sb", name="bc_sb", bufs=1)
                nc.scalar.activation(bc_sb[:], bc_ps[:], AF.Copy)
                nc.vector.tensor_tensor(oT[:, h, :], av_ps[:], bc_sb[:],
                                        op=ALU.mult)

            # -------- M = attn @ out_w.T ; H2 = h1 + M + out_b --------------
            h2 = apool.tile([128, MT, H], bf16, tag="h23", name="h2_sb")
            for m in range(MT):
                ps = psAgg.tile([128, H], f32, tag="agg", name="m_ps")
                for c in range(HEADS):
                    nc.tensor.matmul(ps[:], oT[:, c, 128 * m:128 * (m + 1)],
                                     outwT[:, c, :], start=(c == 0),
                                     stop=(c == HEADS - 1))
                t = wpool.tile([128, H], f32, tag="ep2", name="ep2_t")
                nc.vector.tensor_tensor(t[:], ps[:], h1f[:, m, :], op=ALU.add)
                nc.vector.tensor_tensor(h2[:, m, :], t[:], outbbc[:], op=ALU.add)

            # -------- T2, P2c, AllGather (2 halves) -------------------------
            t2 = apool.tile([128, H // 128, S], bf16, tag="t23", name="t2_sb")
            for c in range(H // 128):
                tp = psSc.tile([128, S], bf16, tag="sc", name="t2_ps")
                for m in range(MT):
                    nc.tensor.transpose(tp[:, 128 * m:128 * (m + 1)],
                                        h2[:, m, 128 * c:128 * (c + 1)], ident[:])
                nc.vector.tensor_copy(t2[:, c, :], tp[:])
            ag2_in = [dram.tile([S // 2, H], bf16, name=f"ag2i{hf}")
                      for hf in range(2)]
            ag2_out = [dram.tile([N // 2, H], bf16, name=f"ag2o{hf}",
                              addr_space="Shared")
                       for hf in range(2)]
            for half in range(2):
                for i in range(2):
                    m = 2 * half + i
                    ps = psAgg.tile([128, H], f32, tag="agg", name="p2_ps")
                    for c in range(H // 128):
                        nc.tensor.matmul(ps[:], t2[:, c, 128 * m:128 * (m + 1)],
                                         w2[:, c, :], start=(c == 0),
                                         stop=(c == H // 128 - 1))
                    p2c = wpool.tile([128, H], bf16, tag="p2c", name="p2c_t")
                    nc.scalar.activation(p2c[:], ps[:], AF.Identity,
                                         scale=dinv_col[:, m:m + 1])
                    nc.sync.dma_start(
                        ag2_in[half][:].rearrange("(mm p) h -> p mm h",
                                                  p=128)[:, i, :],
                        p2c[:])
                nc.gpsimd.collective_compute(
                    "AllGather", ALU.bypass, ins=[ag2_in[half].opt()],
                    outs=[ag2_out[half].opt()],
                    replica_groups=[list(range(NCORES))])

            # -------- conv2 + classifier ------------------------------------
            h3 = apool.tile([128, MT, H], bf16, tag="h23", name="h3_sb")
            for hf in range(2):
                hps = [psAgg.tile([128, H], f32, tag="agg", name=f"h3ps{hf}{i}")
                       for i in range(2)]
                first, last = (0, 0, 0), (1, NCORES - 1, 1)
                for half in range(2):
                    agv2 = ag2_out[half][:].rearrange("(cc p) h -> p cc h", p=128)
                    for cc in range(NCORES):
                        rhs = spool.tile([128, 2, H], bf16, tag="agc2",
                                         name="agc2_t")
                        nc.sync.dma_start(rhs[:],
                                          agv2[:, 2 * cc:2 * (cc + 1), :])
                        for i in range(2):
                            for k in range(2):
                                m = 2 * hf + k
                                nc.tensor.matmul(
                                    hps[k][:],
                                    acT[:, 4 * cc + 2 * half + i,
                                        128 * m:128 * (m + 1)],
                                    rhs[:, i, :],
                                    start=((half, cc, i) == first),
                                    stop=((half, cc, i) == last))
                for k in range(2):
                    m = 2 * hf + k
                    t = wpool.tile([128, H], f32, tag="ep3", name="ep3_t")
                    nc.scalar.activation(t[:], hps[k][:], AF.Identity,
                                         scale=dinv_col[:, m:m + 1])
                    nc.vector.tensor_tensor(t[:], t[:], b2bc[:], op=ALU.add)
                    nc.vector.tensor_scalar_max(h3[:, m, :], t[:], 0.0)

            t3 = apool.tile([128, H // 128, S], bf16, tag="t23", name="t3_sb")
            for c in range(H // 128):
                tp = psSc.tile([128, S], bf16, tag="sc", name="t3_ps")
                for m in range(MT):
                    nc.tensor.transpose(tp[:, 128 * m:128 * (m + 1)],
                                        h3[:, m, 128 * c:128 * (c + 1)], ident[:])
                nc.vector.tensor_copy(t3[:, c, :], tp[:])

            for m in range(MT):
                ps = psAv.tile([128, C], f32, tag="av", name="oc_ps")
                for c in range(H // 128):
                    nc.tensor.matmul(ps[:], t3[:, c, 128 * m:128 * (m + 1)],
                                     wc[:, c, :], start=(c == 0),
                                     stop=(c == H // 128 - 1))
                ot = wpool.tile([128, C], f32, tag="ot", name="ot_t", bufs=1)
                nc.vector.tensor_tensor(ot[:], ps[:], bccbc[:], op=ALU.add)
                nc.sync.dma_start(
                    out_t.ap().rearrange("(mm p) c -> p mm c", p=128)[:, m, :],
                    ot[:])

    nc.compile()
    return nc


def _get_compiled():
    global _compiled
    if _compiled is None:
        _compiled = _build()
    return _compiled


def _prep_inputs(x, edge_index, W1, b1, in_w, in_b, out_w, out_b, W2, b2, Wc, bc):
    x = np.asarray(x, np.float32)
    ei = np.asarray(edge_index, np.int64)
    src, dst = ei[0], ei[1]

    shared = {
        "xT": np.ascontiguousarray(x.T).astype(BF16),
        "w1": np.asarray(W1, np.float32).astype(BF16),
        "inwT": np.ascontiguousarray(np.asarray(in_w, np.float32).T).astype(BF16),
        "outwT": np.ascontiguousarray(np.asarray(out_w, np.float32).T).astype(BF16),
        "w2": np.asarray(W2, np.float32).astype(BF16),
        "wc": np.asarray(Wc, np.float32).astype(BF16),
        "b1bc": np.tile(np.asarray(b1, np.float32)[None, :], (128, 1)),
        "b2bc": np.tile(np.asarray(b2, np.float32)[None, :], (128, 1)),
        "outbbc": np.tile(np.asarray(out_b, np.float32)[None, :], (128, 1)),
        "inbvbc": np.tile(np.asarray(in_b, np.float32)[2 * H:3 * H][None, :],
                          (128, 1)),
        "bccbc": np.tile(np.asarray(bc, np.float32)[None, :], (128, 1)),
        "inbq": np.ascontiguousarray(
            np.asarray(in_b, np.float32)[0:H].reshape(HEADS, 128).T),
        "inbk": np.ascontiguousarray(
            np.asarray(in_b, np.float32)[H:2 * H].reshape(HEADS, 128).T),
        "ident": np.eye(128, dtype=np.float32).astype(BF16),
    }

    in_maps = []
    for k in range(NCORES):
        lo, hi = S * k, S * (k + 1)
        sel = (dst >= lo) & (dst < hi)
        ac = np.zeros((N, S), np.float32)
        np.add.at(ac, (src[sel], dst[sel] - lo), 1.0)
        ac[np.arange(lo, hi), np.arange(S)] += 1.0  # self loops
        m = dict(shared)
        m["acT"] = ac.astype(BF16)
        in_maps.append(m)
    return in_maps


def kernel(**inputs) -> np.ndarray:
    from concourse import bass_utils
    nc = _get_compiled()
    in_maps = _prep_inputs(**inputs)
    res = bass_utils.run_bass_kernel_spmd(nc, in_maps,
                                          core_ids=list(range(NCORES)))
    return np.concatenate([res.results[k]["y"] for k in range(NCORES)], axis=0)


# revision 31
# speedup vs baseline: 1.2423x; 1.1125x over previous
"""EnhancedGCNWithAttention on 8 Trainium2 NeuronCores (Bass/Tile SPMD).

Strategy (node-sharded, 512 nodes per core):
  - GCN aggregation is cast as dense matmuls against a per-core count matrix
    acT[4096 src, 512 local dst] (bf16, exact small integers) built on the
    host from the edge multiset (+ self loops). Degrees, rsqrt normalization,
    and everything else arithmetic run on device.
  - A_norm @ P == diag(dinv_dst) . (acT.T @ (diag(dinv_src) . P)); the source
    scaling is a per-partition ACT scale applied to P tiles, the dst scaling
    rides the aggregation epilogue.
  - x @ W1 is computed for ALL nodes on every core (cheap), so the only
    pre-conv1 communication is an AllGather of the 2KB degree vector.
  - Attention is sharded by query rows. K^T/V are AllGathered per head in 4
    small collectives so head h's compute hides head h+1's gather. Scores
    are computed transposed (keys on partitions); exp row-sums use pairwise
    DVE adds + a ones-vector matmul; 1/sum is applied after attn@V.
"""
import sys

sys.path.insert(0, '/opt/trn_rl_repo')

import numpy as np
import ml_dtypes

N, F_IN, H, HEADS, C, E = 4096, 768, 512, 4, 32, 131072
DH = H // HEADS            # 128
NCORES = 8
S = N // NCORES            # 512 rows per core
SCALE = 1.0 / np.sqrt(DH)

BF16 = ml_dtypes.bfloat16

_compiled = None


def _build():
    from concourse import bacc, tile, mybir

    f32 = mybir.dt.float32
    bf16 = mybir.dt.bfloat16
    AF = mybir.ActivationFunctionType
    ALU = mybir.AluOpType

    NT = N // 128     # 32 node tiles
    MT = S // 128     # 4 local row tiles

    nc = bacc.Bacc("TRN2", target_bir_lowering=False, debug=False,
                   enable_asserts=False, num_devices=NCORES)

    # ---------------- I/O ----------------
    acT_in = nc.dram_tensor("acT", [N, S], bf16, kind="ExternalInput")
    xT_in = nc.dram_tensor("xT", [F_IN, N], bf16, kind="ExternalInput")
    w1_in = nc.dram_tensor("w1", [F_IN, H], bf16, kind="ExternalInput")
    inwT_in = nc.dram_tensor("inwT", [H, 3 * H], bf16, kind="ExternalInput")
    outwT_in = nc.dram_tensor("outwT", [H, H], bf16, kind="ExternalInput")
    w2_in = nc.dram_tensor("w2", [H, H], bf16, kind="ExternalInput")
    wc_in = nc.dram_tensor("wc", [H, C], bf16, kind="ExternalInput")
    b1bc_in = nc.dram_tensor("b1bc", [128, H], f32, kind="ExternalInput")
    b2bc_in = nc.dram_tensor("b2bc", [128, H], f32, kind="ExternalInput")
    outbbc_in = nc.dram_tensor("outbbc", [128, H], f32, kind="ExternalInput")
    inbvbc_in = nc.dram_tensor("inbvbc", [128, H], f32, kind="ExternalInput")
    bccbc_in = nc.dram_tensor("bccbc", [128, C], f32, kind="ExternalInput")
    inbq_in = nc.dram_tensor("inbq", [128, HEADS], f32, kind="ExternalInput")
    inbk_in = nc.dram_tensor("inbk", [128, HEADS], f32, kind="ExternalInput")
    ident_in = nc.dram_tensor("ident", [128, 128], bf16, kind="ExternalInput")
    out_t = nc.dram_tensor("y", [S, C], f32, kind="ExternalOutput")

    with tile.TileContext(nc) as tc:
        with tc.tile_pool(name="const", bufs=1) as cpool, \
             tc.tile_pool(name="acts", bufs=1) as apool, \
             tc.tile_pool(name="stream", bufs=4) as spool, \
             tc.tile_pool(name="kv", bufs=2) as kvpool, \
             tc.tile_pool(name="work", bufs=2) as wpool, \
             tc.tile_pool(name="psAgg", bufs=2, space="PSUM") as psAgg, \
             tc.tile_pool(name="psSc", bufs=2, space="PSUM") as psSc, \
             tc.tile_pool(name="psAv", bufs=1, space="PSUM") as psAv, \
             tc.tile_pool(name="psSum", bufs=1, space="PSUM") as psSum, \
             tc.tile_pool(name="dram", bufs=1, space="DRAM") as dram:

            # -------- constant loads --------
            def ld(name, t_in, shape, dt, pool=cpool, view=None):
                t = pool.tile(shape, dt, tag=name, name=name + "_sb")
                src = t_in.ap() if view is None else view
                nc.sync.dma_start(t[:], src)
                return t

            w1 = ld("w1", w1_in, [128, F_IN // 128, H], bf16,
                    view=w1_in.ap().rearrange("(c p) h -> p c h", p=128))
            inwT = ld("inwT", inwT_in, [128, H // 128, 3 * H], bf16,
                      view=inwT_in.ap().rearrange("(c p) h -> p c h", p=128))
            outwT = ld("outwT", outwT_in, [128, H // 128, H], bf16,
                       view=outwT_in.ap().rearrange("(c p) h -> p c h", p=128))
            w2 = ld("w2", w2_in, [128, H // 128, H], bf16,
                    view=w2_in.ap().rearrange("(c p) h -> p c h", p=128))
            wc = ld("wc", wc_in, [128, H // 128, C], bf16,
                    view=wc_in.ap().rearrange("(c p) h -> p c h", p=128))
            b1bc = ld("b1bc", b1bc_in, [128, H], f32)
            b2bc = ld("b2bc", b2bc_in, [128, H], f32)
            outbbc = ld("outbbc", outbbc_in, [128, H], f32)
            inbvbc = ld("inbvbc", inbvbc_in, [128, H], f32)
            bccbc = ld("bccbc", bccbc_in, [128, C], f32)
            inbq = ld("inbq", inbq_in, [128, HEADS], f32)
            inbk = ld("inbk", inbk_in, [128, HEADS], f32)
            ident = ld("ident", ident_in, [128, 128], bf16)

            ones_bf = cpool.tile([128, 1], bf16, tag="ones_bf", name="ones_bf")
            nc.vector.memset(ones_bf[:], 1.0)
            ones_row = cpool.tile([1, 128], f32, tag="ones_row", name="ones_row")
            nc.vector.memset(ones_row[:], 1.0)
            one_11 = cpool.tile([1, 1], f32, tag="one_11", name="one_11")
            nc.vector.memset(one_11[:], 1.0)

            # -------- acT load + degrees + dinv + AllGather(dinv) -----------
            acT = cpool.tile([128, NT, S], bf16, tag="acT", name="acT_sb")
            acT_v = acT_in.ap().rearrange("(c p) s -> p c s", p=128)
            for g in range(4):
                nc.sync.dma_start(acT[:, 8 * g:8 * (g + 1), :],
                                  acT_v[:, 8 * g:8 * (g + 1), :])
            deg_ps = psSum.tile([1, S], f32, tag="sum", name="deg_ps")
            for c in range(NT):
                nc.tensor.matmul(deg_ps[:], ones_bf[:], acT[:, c, :],
                                 start=(c == 0), stop=(c == NT - 1))
            rdeg = cpool.tile([1, S], f32, tag="rdeg", name="rdeg")
            nc.vector.reciprocal(rdeg[:], deg_ps[:])
            dinv_row = cpool.tile([1, S], f32, tag="dinv_row", name="dinv_row")
            nc.scalar.activation(dinv_row[:], rdeg[:], AF.Sqrt)
            dinv_col = cpool.tile([128, MT], f32, tag="dinv_col", name="dinv_col")
            for m in range(MT):
                dc_ps = psAv.tile([128, 1], f32, tag="av", name="dc_ps")
                nc.tensor.matmul(dc_ps[:], dinv_row[:, 128 * m:128 * (m + 1)],
                                 one_11[:], start=True, stop=True)
                nc.scalar.activation(dinv_col[:, m:m + 1], dc_ps[:], AF.Copy)
            agd_in = dram.tile([1, S], f32, name="agd_in")
            agd_out = dram.tile([NCORES, S], f32, name="agd_out",
                                addr_space="Shared")
            nc.sync.dma_start(agd_in[:], dinv_row[:])
            nc.gpsimd.collective_compute(
                "AllGather", ALU.bypass, ins=[agd_in.opt()],
                outs=[agd_out.opt()], replica_groups=[list(range(NCORES))])
            # -------- p1 = x @ W1 for ALL nodes (fills the launch-skew window)
            p1c = [apool.tile([128, H], bf16, tag=f"p1c{t}",
                               name=f"p1c{t}") for t in range(NT)]
            xT_v = xT_in.ap().rearrange("(c p) (t q) -> p t c q", p=128, q=128)
            for t in range(NT):
                xt_t = spool.tile([128, F_IN // 128, 128], bf16, tag="xt",
                                  name="xt_t")
                nc.sync.dma_start(xt_t[:], xT_v[:, t, :, :])
                ps = psAgg.tile([128, H], f32, tag="agg", name="p1_ps")
                for c in range(F_IN // 128):
                    nc.tensor.matmul(ps[:], xt_t[:, c, :], w1[:, c, :],
                                     start=(c == 0), stop=(c == F_IN // 128 - 1))
                nc.scalar.activation(p1c[t][:], ps[:], AF.Copy)

            # dinv_colf[p, 4c + t] = dinv_full[512c + 128t + p]
            agd_sb = cpool.tile([NCORES, S], f32, tag="agd_sb", name="agd_sb")
            nc.sync.dma_start(agd_sb[:], agd_out[:])
            dinv_colf = cpool.tile([128, NT], f32, tag="dinv_colf",
                                   name="dinv_colf")
            identf = cpool.tile([128, 128], f32, tag="identf", name="identf")
            nc.vector.tensor_copy(identf[:], ident[:])
            for t in range(MT):
                tps = psSc.tile([128, NCORES], f32, tag="sc", name="dct_ps")
                nc.tensor.transpose(tps[:], agd_sb[:, 128 * t:128 * (t + 1)],
                                    identf[0:NCORES, 0:NCORES])
                nc.vector.tensor_copy(
                    dinv_colf[:].rearrange("p (c t) -> p c t", t=MT)[:, :, t],
                    tps[:])
            # fold the source-side dinv into the p1c tiles (per chunk)
            for t in range(NT):
                nc.vector.tensor_scalar_mul(p1c[t][:], p1c[t][:],
                                            dinv_colf[:, t:t + 1])

            # -------- conv1: single pass, rhs from SBUF ---------------------
            h1f = apool.tile([128, MT, H], f32, tag="h1f", name="h1f_sb")
            hps1 = [psAgg.tile([128, H], f32, tag="agg", name=f"cv1{i}")
                    for i in range(2)]
            hps1 += [psSc.tile([128, H], f32, tag="sc", name=f"cv1{i + 2}")
                     for i in range(2)]
            for c in range(NT):
                for m in range(MT):
                    nc.tensor.matmul(hps1[m][:],
                                     acT[:, c, 128 * m:128 * (m + 1)],
                                     p1c[c][:], start=(c == 0),
                                     stop=(c == NT - 1))
            for m in range(MT):
                t = wpool.tile([128, H], f32, tag="ep1", name="ep_t")
                nc.scalar.activation(t[:], hps1[m][:], AF.Identity,
                                     scale=dinv_col[:, m:m + 1])
                nc.vector.tensor_tensor(t[:], t[:], b1bc[:], op=ALU.add)
                nc.vector.tensor_scalar_max(h1f[:, m, :], t[:], 0.0)

            # -------- T1 = h1^T --------
            t1 = apool.tile([128, H // 128, S], bf16, tag="t1", name="t1_sb")
            for c in range(H // 128):
                tp = psSc.tile([128, S], f32, tag="sc", name="t1_ps")
                for m in range(MT):
                    nc.tensor.transpose(tp[:, 128 * m:128 * (m + 1)],
                                        h1f[:, m, 128 * c:128 * (c + 1)],
                                        identf[:])
                nc.vector.tensor_copy(t1[:, c, :], tp[:])

            # -------- qkv: V, then K^T head 0 + AG0, Q, K^T heads 1-3 -------
            agkv_in = [dram.tile([2 * DH, S], bf16, name=f"agkvi{h}")
                       for h in range(HEADS)]
            agkv_out = [dram.tile([NCORES * 2 * DH, S], bf16, name=f"agkvo{h}",
                                  addr_space="Shared")
                        for h in range(HEADS)]
            vv = apool.tile([128, MT, H], bf16, tag="vvqt", name="vv_sb")
            for m in range(MT):
                ps = psAgg.tile([128, H], f32, tag="agg", name="v_ps")
                for c in range(H // 128):
                    nc.tensor.matmul(ps[:], t1[:, c, 128 * m:128 * (m + 1)],
                                     inwT[:, c, 2 * H:3 * H], start=(c == 0),
                                     stop=(c == H // 128 - 1))
                nc.vector.tensor_tensor(vv[:, m, :], ps[:], inbvbc[:], op=ALU.add)

            kt = apool.tile([128, HEADS, S], bf16, tag="kt", name="kt_sb")

            def k_head(h):
                ps2 = psAgg.tile([128, S], f32, tag="agg", name="k_ps")
                for c in range(H // 128):
                    nc.tensor.matmul(ps2[:],
                                     inwT[:, c, H + 128 * h:H + 128 * (h + 1)],
                                     t1[:, c, :], start=(c == 0),
                                     stop=(c == H // 128 - 1))
                nc.vector.tensor_scalar_add(kt[:, h, :], ps2[:], inbk[:, h:h + 1])
                nc.sync.dma_start(agkv_in[h][0:DH, :], kt[:, h, :])
                nc.sync.dma_start(
                    agkv_in[h][DH:2 * DH, :].rearrange("p (m v) -> p m v", m=MT),
                    vv[:, :, 128 * h:128 * (h + 1)])
                nc.gpsimd.collective_compute(
                    "AllGather", ALU.bypass, ins=[agkv_in[h].opt()],
                    outs=[agkv_out[h].opt()],
                    replica_groups=[list(range(NCORES))])

            k_head(0)

            qt = apool.tile([128, HEADS, S], bf16, tag="vvqt", name="qt_sb")
            inbq_s = cpool.tile([128, HEADS], f32, tag="inbq_s", name="inbq_s")
            nc.vector.tensor_scalar_mul(inbq_s[:], inbq[:], SCALE)
            for h in range(HEADS):
                ps = psAgg.tile([128, S], f32, tag="agg", name="q_ps")
                for c in range(H // 128):
                    nc.tensor.matmul(ps[:], inwT[:, c, 128 * h:128 * (h + 1)],
                                     t1[:, c, :], start=(c == 0),
                                     stop=(c == H // 128 - 1))
                nc.vector.tensor_scalar(qt[:, h, :], ps[:], SCALE,
                                        inbq_s[:, h:h + 1], op0=ALU.mult,
                                        op1=ALU.add)
            for h in range(1, HEADS):
                k_head(h)

            # -------- attention, heads pipelined over their AllGathers ------
            oT = apool.tile([128, HEADS, S], bf16, tag="oT", name="oT_sb")
            pending_epi = []

            def head_epilogue(h, av_ps, sum_ps):
                rsum = wpool.tile([1, S], f32, tag="rsum", name="rsum_t",
                                  bufs=2)
                nc.vector.reciprocal(rsum[:], sum_ps[:])
                bc_ps = psSc.tile([128, S], f32, tag="sc", name="bc_ps")
                nc.tensor.matmul(bc_ps[:], ones_row[:], rsum[:], start=True,
                                 stop=True)
                bc_sb = wpool.tile([128, S], f32, tag="bc_sb", name="bc_sb",
                                   bufs=1)
                nc.scalar.activation(bc_sb[:], bc_ps[:], AF.Copy)
                nc.vector.tensor_tensor(oT[:, h, :], av_ps[:], bc_sb[:],
                                        op=ALU.mult)

            for h in range(HEADS):
                agv = agkv_out[h][:].rearrange("(c z p) s -> p z c s", p=128, z=2)
                kth = kvpool.tile([128, NCORES, S], bf16, tag="kth", name="kth")
                nc.sync.dma_start(kth[:], agv[:, 0, :, :])
                vh = kvpool.tile([128, NT, DH], bf16, tag="vh", name="vh")
                nc.sync.dma_start(
                    vh[:].rearrange("p (c t) v -> p c t v", c=NCORES),
                    agv[:, 1, :, :].rearrange("p c (t v) -> p c t v", t=MT))
                kthf = kth[:].rearrange("p c s -> p (c s)")
                av_ps = psAv.tile([128, S], f32, tag="av", name="av_ps")
                sum_ps = psSum.tile([1, S], f32, tag="sum", name="sum_ps")
                l1_prev = None
                deferred = []
                for g in range(NT // 2):            # 2 key blocks per group
                    sca = psSc.tile([128, 2, S], f32, tag="sc", name="sca_ps")
                    for j in range(2):
                        b = 2 * g + j
                        nc.tensor.matmul(sca[:, j, :],
                                         kthf[:, 128 * b:128 * (b + 1)],
                                         qt[:, h, :], start=True, stop=True)
                    ex = wpool.tile([128, 2, S], bf16, tag="ex", name="ex_t", bufs=2)
                    nc.scalar.activation(ex[:], sca[:], AF.Exp)

                    def tail(g=g, ex=ex):
                        nonlocal l1_prev
                        for j in range(2):
                            nc.tensor.matmul(av_ps[:], vh[:, 2 * g + j, :],
                                             ex[:, j, :],
                                             start=(g == 0 and j == 0),
                                             stop=(g == NT // 2 - 1 and j == 1))
                        l1 = wpool.tile([128, S], bf16, tag="l1", name="l1_t")
                        nc.vector.tensor_tensor(l1[:], ex[:, 0, :], ex[:, 1, :],
                                                op=ALU.add)
                        if g % 2 == 0:
                            l1_prev = l1
                        else:
                            l2 = wpool.tile([128, S], bf16, tag="l2",
                                            name="l2_t", bufs=1)
                            nc.vector.tensor_tensor(l2[:], l1_prev[:], l1[:],
                                                    op=ALU.add)
                            nc.tensor.matmul(sum_ps[:], ones_bf[:], l2[:],
                                             start=(g == 1),
                                             stop=(g == NT // 2 - 1))
                    if g < 2:
                        deferred.append(tail)
                    else:
                        if g == 2:
                            if pending_epi:
                                pending_epi.pop()()
                            for fn in deferred:
                                fn()
                            deferred = []
                        tail()
                pending_epi.append(
                    lambda h=h, a=av_ps, s=sum_ps: head_epilogue(h, a, s))
            pending_epi.pop()()

            # -------- M = attn @ out_w.T ; H2 = h1 + M + out_b --------------
            h2 = apool.tile([128, MT, H], bf16, tag="h23", name="h2_sb")
            for m in range(MT):
                ps = psAgg.tile([128, H], f32, tag="agg", name="m_ps")
                for c in range(HEADS):
                    nc.tensor.matmul(ps[:], oT[:, c, 128 * m:128 * (m + 1)],
                                     outwT[:, c, :], start=(c == 0),
                                     stop=(c == HEADS - 1))
                t = wpool.tile([128, H], f32, tag="ep2", name="ep2_t")
                nc.vector.tensor_tensor(t[:], ps[:], h1f[:, m, :], op=ALU.add)
                nc.vector.tensor_tensor(h2[:, m, :], t[:], outbbc[:], op=ALU.add)

            # -------- T2, P2 (unscaled), AllGather halves -------------------
            t2 = apool.tile([128, H // 128, S], bf16, tag="t23", name="t2_sb")
            for c in range(H // 128):
                tp = psSc.tile([128, S], bf16, tag="sc", name="t2_ps")
                for m in range(MT):
                    nc.tensor.transpose(tp[:, 128 * m:128 * (m + 1)],
                                        h2[:, m, 128 * c:128 * (c + 1)], ident[:])
                nc.vector.tensor_copy(t2[:, c, :], tp[:])
            ag2_in = [dram.tile([2 * 128, H], bf16, name=f"ag2i{hf}")
                      for hf in range(2)]
            ag2_out = [dram.tile([NCORES * 2 * 128, H], bf16, name=f"ag2o{hf}",
                                 addr_space="Shared")
                       for hf in range(2)]
            for half in range(2):
                for i in range(2):
                    m = 2 * half + i
                    ps = psAgg.tile([128, H], f32, tag="agg", name="p2_ps")
                    for c in range(H // 128):
                        nc.tensor.matmul(ps[:], t2[:, c, 128 * m:128 * (m + 1)],
                                         w2[:, c, :], start=(c == 0),
                                         stop=(c == H // 128 - 1))
                    p2c = wpool.tile([128, H], bf16, tag="p2c", name="p2c_t")
                    nc.scalar.activation(p2c[:], ps[:], AF.Identity,
                                         scale=dinv_col[:, m:m + 1])
                    nc.sync.dma_start(
                        ag2_in[half][:].rearrange("(mm p) h -> p mm h",
                                                  p=128)[:, i, :],
                        p2c[:])
                nc.gpsimd.collective_compute(
                    "AllGather", ALU.bypass, ins=[ag2_in[half].opt()],
                    outs=[ag2_out[half].opt()],
                    replica_groups=[list(range(NCORES))])

            # -------- conv2: single pass, rhs streamed from the AG halves ---
            h3 = apool.tile([128, MT, H], bf16, tag="h23", name="h3_sb")
            hps2 = [psAgg.tile([128, H], f32, tag="agg", name=f"cv2{i}")
                    for i in range(2)]
            hps2 += [psSc.tile([128, H], f32, tag="sc", name=f"cv2{i + 2}")
                     for i in range(2)]
            for half in range(2):
                agv2 = ag2_out[half][:].rearrange("(cc p) h -> p cc h", p=128)
                for cc in range(NCORES):
                    rhs = spool.tile([128, 2, H], bf16, tag="agc", name="agc_t")
                    nc.sync.dma_start(rhs[:], agv2[:, 2 * cc:2 * (cc + 1), :])
                    for i in range(2):
                        for m in range(MT):
                            nc.tensor.matmul(
                                hps2[m][:],
                                acT[:, 4 * cc + 2 * half + i,
                                    128 * m:128 * (m + 1)],
                                rhs[:, i, :],
                                start=(half == 0 and cc == 0 and i == 0),
                                stop=(half == 1 and cc == NCORES - 1 and i == 1))
            for m in range(MT):
                t = wpool.tile([128, H], f32, tag="ep1", name="ep3_t")
                nc.scalar.activation(t[:], hps2[m][:], AF.Identity,
                                     scale=dinv_col[:, m:m + 1])
                nc.vector.tensor_tensor(t[:], t[:], b2bc[:], op=ALU.add)
                nc.vector.tensor_scalar_max(h3[:, m, :], t[:], 0.0)

            t3 = apool.tile([128, H // 128, S], bf16, tag="t23", name="t3_sb")
            for c in range(H // 128):
                tp = psSc.tile([128, S], bf16, tag="sc", name="t3_ps")
                for m in range(MT):
                    nc.tensor.transpose(tp[:, 128 * m:128 * (m + 1)],
                                        h3[:, m, 128 * c:128 * (c + 1)], ident[:])
                nc.vector.tensor_copy(t3[:, c, :], tp[:])

            for m in range(MT):
                ps = psAv.tile([128, C], f32, tag="av", name="oc_ps")
                for c in range(H // 128):
                    nc.tensor.matmul(ps[:], t3[:, c, 128 * m:128 * (m + 1)],
                                     wc[:, c, :], start=(c == 0),
                                     stop=(c == H // 128 - 1))
                ot = wpool.tile([128, C], f32, tag="ot", name="ot_t", bufs=1)
                nc.vector.tensor_tensor(ot[:], ps[:], bccbc[:], op=ALU.add)
                nc.sync.dma_start(
                    out_t.ap().rearrange("(mm p) c -> p mm c", p=128)[:, m, :],
                    ot[:])

    nc.compile()
    return nc


def _get_compiled():
    global _compiled
    if _compiled is None:
        _compiled = _build()
    return _compiled


def _prep_inputs(x, edge_index, W1, b1, in_w, in_b, out_w, out_b, W2, b2, Wc, bc):
    x = np.asarray(x, np.float32)
    ei = np.asarray(edge_index, np.int64)
    src, dst = ei[0], ei[1]

    shared = {
        "xT": np.ascontiguousarray(x.T).astype(BF16),
        "w1": np.asarray(W1, np.float32).astype(BF16),
        "inwT": np.ascontiguousarray(np.asarray(in_w, np.float32).T).astype(BF16),
        "outwT": np.ascontiguousarray(np.asarray(out_w, np.float32).T).astype(BF16),
        "w2": np.asarray(W2, np.float32).astype(BF16),
        "wc": np.asarray(Wc, np.float32).astype(BF16),
        "b1bc": np.tile(np.asarray(b1, np.float32)[None, :], (128, 1)),
        "b2bc": np.tile(np.asarray(b2, np.float32)[None, :], (128, 1)),
        "outbbc": np.tile(np.asarray(out_b, np.float32)[None, :], (128, 1)),
        "inbvbc": np.tile(np.asarray(in_b, np.float32)[2 * H:3 * H][None, :],
                          (128, 1)),
        "bccbc": np.tile(np.asarray(bc, np.float32)[None, :], (128, 1)),
        "inbq": np.ascontiguousarray(
            np.asarray(in_b, np.float32)[0:H].reshape(HEADS, 128).T),
        "inbk": np.ascontiguousarray(
            np.asarray(in_b, np.float32)[H:2 * H].reshape(HEADS, 128).T),
        "ident": np.eye(128, dtype=np.float32).astype(BF16),
    }

    in_maps = []
    for k in range(NCORES):
        lo, hi = S * k, S * (k + 1)
        sel = (dst >= lo) & (dst < hi)
        ac = np.zeros((N, S), np.float32)
        np.add.at(ac, (src[sel], dst[sel] - lo), 1.0)
        ac[np.arange(lo, hi), np.arange(S)] += 1.0  # self loops
        m = dict(shared)
        m["acT"] = ac.astype(BF16)
        in_maps.append(m)
    return in_maps


def _run_device(in_maps):
    from concourse import bass_utils
    nc = _get_compiled()
    res = bass_utils.run_bass_kernel_spmd(nc, in_maps,
                                          core_ids=list(range(NCORES)))
    return np.concatenate([res.results[k]["y"] for k in range(NCORES)], axis=0)


def _run_subprocess(in_maps):
    """Fresh-process retry: survives a wedged NRT/axon client in this one."""
    import os
    import subprocess
    import tempfile
    d = tempfile.mkdtemp(prefix="gcnkrn_")
    fin = os.path.join(d, "in.npz")
    fout = os.path.join(d, "out.npy")
    np.savez(fin, **{f"c{k}_{n}": a for k, m in enumerate(in_maps)
                     for n, a in m.items()})
    env = {k: v for k, v in os.environ.items() if k != "JAX_PLATFORMS"}
    subprocess.run([sys.executable, os.path.abspath(__file__), fin, fout],
                   check=True, timeout=900, env=env)
    return np.load(fout)


def kernel(**inputs) -> np.ndarray:
    in_maps = _prep_inputs(**inputs)
    try:
        return _run_device(in_maps)
    except Exception:
        err = None
        for _ in range(2):
            try:
                return _run_subprocess(in_maps)
            except Exception as e:
                err = e
        raise err


if __name__ == "__main__":
    _fin, _fout = sys.argv[1], sys.argv[2]
    _z = np.load(_fin)
    _maps = [{} for _ in range(NCORES)]
    _bf_names = {"acT", "xT", "w1", "inwT", "outwT", "w2", "wc", "ident"}
    for _key in _z.files:
        _ck, _name = _key.split("_", 1)
        _a = _z[_key]
        if _a.dtype == np.dtype("V2") or _name in _bf_names:
            _a = _a.view(BF16) if _a.dtype.itemsize == 2 else _a.astype(BF16)
        _maps[int(_ck[1:])][_name] = _a
    np.save(_fout, _run_device(_maps))


# revision 33
# speedup vs baseline: 1.2934x; 1.0411x over previous
"""EnhancedGCNWithAttention on 8 Trainium2 NeuronCores (Bass/Tile SPMD).

Strategy (node-sharded, 512 nodes per core):
  - GCN aggregation is cast as dense matmuls against a per-core count matrix
    acT[4096 src, 512 local dst] (bf16, exact small integers) built on the
    host from the edge multiset (+ self loops). Degrees, rsqrt normalization,
    and everything else arithmetic run on device.
  - A_norm @ P == diag(dinv_dst) . (acT.T @ (diag(dinv_src) . P)); the source
    scaling is a per-partition ACT scale applied to P tiles, the dst scaling
    rides the aggregation epilogue.
  - x @ W1 is computed for ALL nodes on every core (cheap), so the only
    pre-conv1 communication is an AllGather of the 2KB degree vector.
  - Attention is sharded by query rows. K^T/V are AllGathered per head in 4
    small collectives so head h's compute hides head h+1's gather. Scores
    are computed transposed (keys on partitions); exp row-sums use pairwise
    DVE adds + a ones-vector matmul; 1/sum is applied after attn@V.
"""
import sys

sys.path.insert(0, '/opt/trn_rl_repo')

import numpy as np
import ml_dtypes

N, F_IN, H, HEADS, C, E = 4096, 768, 512, 4, 32, 131072
DH = H // HEADS            # 128
NCORES = 8
S = N // NCORES            # 512 rows per core
SCALE = 1.0 / np.sqrt(DH)

BF16 = ml_dtypes.bfloat16

_compiled = None


def _build():
    from concourse import bacc, tile, mybir

    f32 = mybir.dt.float32
    bf16 = mybir.dt.bfloat16
    AF = mybir.ActivationFunctionType
    ALU = mybir.AluOpType

    NT = N // 128     # 32 node tiles
    MT = S // 128     # 4 local row tiles

    nc = bacc.Bacc("TRN2", target_bir_lowering=False, debug=False,
                   enable_asserts=False, num_devices=NCORES)

    # ---------------- I/O ----------------
    acT_in = nc.dram_tensor("acT", [N, S], bf16, kind="ExternalInput")
    xT_in = nc.dram_tensor("xT", [F_IN, N], bf16, kind="ExternalInput")
    w1_in = nc.dram_tensor("w1", [F_IN, H], bf16, kind="ExternalInput")
    inwT_in = nc.dram_tensor("inwT", [H, 3 * H], bf16, kind="ExternalInput")
    outwT_in = nc.dram_tensor("outwT", [H, H], bf16, kind="ExternalInput")
    w2_in = nc.dram_tensor("w2", [H, H], bf16, kind="ExternalInput")
    wc_in = nc.dram_tensor("wc", [H, C], bf16, kind="ExternalInput")
    b1bc_in = nc.dram_tensor("b1bc", [128, H], f32, kind="ExternalInput")
    b2bc_in = nc.dram_tensor("b2bc", [128, H], f32, kind="ExternalInput")
    outbbc_in = nc.dram_tensor("outbbc", [128, H], f32, kind="ExternalInput")
    inbvbc_in = nc.dram_tensor("inbvbc", [128, H], f32, kind="ExternalInput")
    bccbc_in = nc.dram_tensor("bccbc", [128, C], f32, kind="ExternalInput")
    inbq_in = nc.dram_tensor("inbq", [128, HEADS], f32, kind="ExternalInput")
    inbk_in = nc.dram_tensor("inbk", [128, HEADS], f32, kind="ExternalInput")
    ident_in = nc.dram_tensor("ident", [128, 128], bf16, kind="ExternalInput")
    out_t = nc.dram_tensor("y", [S, C], f32, kind="ExternalOutput")

    with tile.TileContext(nc) as tc:
        with tc.tile_pool(name="const", bufs=1) as cpool, \
             tc.tile_pool(name="acts", bufs=1) as apool, \
             tc.tile_pool(name="stream", bufs=4) as spool, \
             tc.tile_pool(name="kv", bufs=2) as kvpool, \
             tc.tile_pool(name="work", bufs=2) as wpool, \
             tc.tile_pool(name="psAgg", bufs=2, space="PSUM") as psAgg, \
             tc.tile_pool(name="psSc", bufs=2, space="PSUM") as psSc, \
             tc.tile_pool(name="psAv", bufs=1, space="PSUM") as psAv, \
             tc.tile_pool(name="psSum", bufs=1, space="PSUM") as psSum, \
             tc.tile_pool(name="dram", bufs=1, space="DRAM") as dram:

            # -------- constant loads --------
            def ld(name, t_in, shape, dt, pool=cpool, view=None):
                t = pool.tile(shape, dt, tag=name, name=name + "_sb")
                src = t_in.ap() if view is None else view
                nc.sync.dma_start(t[:], src)
                return t

            w1 = ld("w1", w1_in, [128, F_IN // 128, H], bf16,
                    view=w1_in.ap().rearrange("(c p) h -> p c h", p=128))
            inwT = ld("inwT", inwT_in, [128, H // 128, 3 * H], bf16,
                      view=inwT_in.ap().rearrange("(c p) h -> p c h", p=128))
            outwT = ld("outwT", outwT_in, [128, H // 128, H], bf16,
                       view=outwT_in.ap().rearrange("(c p) h -> p c h", p=128))
            w2 = ld("w2", w2_in, [128, H // 128, H], bf16,
                    view=w2_in.ap().rearrange("(c p) h -> p c h", p=128))
            wc = ld("wc", wc_in, [128, H // 128, C], bf16,
                    view=wc_in.ap().rearrange("(c p) h -> p c h", p=128))
            b1bc = ld("b1bc", b1bc_in, [128, H], f32)
            b2bc = ld("b2bc", b2bc_in, [128, H], f32)
            outbbc = ld("outbbc", outbbc_in, [128, H], f32)
            inbvbc = ld("inbvbc", inbvbc_in, [128, H], f32)
            bccbc = ld("bccbc", bccbc_in, [128, C], f32)
            inbq = ld("inbq", inbq_in, [128, HEADS], f32)
            inbk = ld("inbk", inbk_in, [128, HEADS], f32)
            ident = ld("ident", ident_in, [128, 128], bf16)

            ones_bf = cpool.tile([128, 1], bf16, tag="ones_bf", name="ones_bf")
            nc.vector.memset(ones_bf[:], 1.0)
            ones_row = cpool.tile([1, 128], f32, tag="ones_row", name="ones_row")
            nc.vector.memset(ones_row[:], 1.0)
            one_11 = cpool.tile([1, 1], f32, tag="one_11", name="one_11")
            nc.vector.memset(one_11[:], 1.0)
            warm = cpool.tile([1, 1], f32, tag="warm", name="warm")
            nc.scalar.activation(warm[:], one_11[:], AF.Exp)

            # -------- acT load + degrees + dinv + AllGather(dinv) -----------
            acT = cpool.tile([128, NT, S], bf16, tag="acT", name="acT_sb")
            acT_v = acT_in.ap().rearrange("(c p) s -> p c s", p=128)
            for g in range(4):
                nc.sync.dma_start(acT[:, 8 * g:8 * (g + 1), :],
                                  acT_v[:, 8 * g:8 * (g + 1), :])
            deg_ps = psSum.tile([1, S], f32, tag="sum", name="deg_ps")
            for c in range(NT):
                nc.tensor.matmul(deg_ps[:], ones_bf[:], acT[:, c, :],
                                 start=(c == 0), stop=(c == NT - 1))
            rdeg = cpool.tile([1, S], f32, tag="rdeg", name="rdeg")
            nc.vector.reciprocal(rdeg[:], deg_ps[:])
            dinv_row = cpool.tile([1, S], f32, tag="dinv_row", name="dinv_row")
            nc.scalar.activation(dinv_row[:], rdeg[:], AF.Sqrt)
            dinv_col = cpool.tile([128, MT], f32, tag="dinv_col", name="dinv_col")
            for m in range(MT):
                dc_ps = psAv.tile([128, 1], f32, tag="av", name="dc_ps")
                nc.tensor.matmul(dc_ps[:], dinv_row[:, 128 * m:128 * (m + 1)],
                                 one_11[:], start=True, stop=True)
                nc.scalar.activation(dinv_col[:, m:m + 1], dc_ps[:], AF.Copy)
            agd_in = dram.tile([1, S], f32, name="agd_in")
            agd_out = dram.tile([NCORES, S], f32, name="agd_out",
                                addr_space="Shared")
            nc.sync.dma_start(agd_in[:], dinv_row[:])
            nc.gpsimd.collective_compute(
                "AllGather", ALU.bypass, ins=[agd_in.opt()],
                outs=[agd_out.opt()], replica_groups=[list(range(NCORES))])
            # -------- p1 = x @ W1 for ALL nodes (fills the launch-skew window)
            p1c = [apool.tile([128, H], bf16, tag=f"p1c{t}",
                               name=f"p1c{t}") for t in range(NT)]
            xT_v = xT_in.ap().rearrange("(c p) (t q) -> p t c q", p=128, q=128)
            for t in range(NT):
                xt_t = spool.tile([128, F_IN // 128, 128], bf16, tag="xt",
                                  name="xt_t")
                nc.sync.dma_start(xt_t[:], xT_v[:, t, :, :])
                ps = psAgg.tile([128, H], f32, tag="agg", name="p1_ps")
                for c in range(F_IN // 128):
                    nc.tensor.matmul(ps[:], xt_t[:, c, :], w1[:, c, :],
                                     start=(c == 0), stop=(c == F_IN // 128 - 1))
                nc.scalar.activation(p1c[t][:], ps[:], AF.Copy)

            # dinv_colf[p, 4c + t] = dinv_full[512c + 128t + p]
            agd_sb = cpool.tile([NCORES, S], f32, tag="agd_sb", name="agd_sb")
            nc.sync.dma_start(agd_sb[:], agd_out[:])
            dinv_colf = cpool.tile([128, NT], f32, tag="dinv_colf",
                                   name="dinv_colf")
            identf = cpool.tile([128, 128], f32, tag="identf", name="identf")
            nc.vector.tensor_copy(identf[:], ident[:])
            for t in range(MT):
                tps = psSc.tile([128, NCORES], f32, tag="sc", name="dct_ps")
                nc.tensor.transpose(tps[:], agd_sb[:, 128 * t:128 * (t + 1)],
                                    identf[0:NCORES, 0:NCORES])
                nc.vector.tensor_copy(
                    dinv_colf[:].rearrange("p (c t) -> p c t", t=MT)[:, :, t],
                    tps[:])
            # fold the source-side dinv into the p1c tiles (per chunk)
            for t in range(NT):
                nc.vector.tensor_scalar_mul(p1c[t][:], p1c[t][:],
                                            dinv_colf[:, t:t + 1])

            # -------- conv1: single pass, rhs from SBUF ---------------------
            h1f = apool.tile([128, MT, H], f32, tag="h1f", name="h1f_sb")
            hps1 = [psAgg.tile([128, H], f32, tag="agg", name=f"cv1{i}")
                    for i in range(2)]
            hps1 += [psSc.tile([128, H], f32, tag="sc", name=f"cv1{i + 2}")
                     for i in range(2)]
            for c in range(NT):
                for m in range(MT):
                    nc.tensor.matmul(hps1[m][:],
                                     acT[:, c, 128 * m:128 * (m + 1)],
                                     p1c[c][:], start=(c == 0),
                                     stop=(c == NT - 1))
            for m in range(MT):
                t = wpool.tile([128, H], f32, tag="ep1", name="ep_t")
                nc.scalar.activation(t[:], hps1[m][:], AF.Identity,
                                     scale=dinv_col[:, m:m + 1])
                nc.vector.tensor_tensor(t[:], t[:], b1bc[:], op=ALU.add)
                nc.vector.tensor_scalar_max(h1f[:, m, :], t[:], 0.0)

            # -------- T1 = h1^T --------
            t1 = apool.tile([128, H // 128, S], bf16, tag="t1", name="t1_sb")
            for c in range(H // 128):
                tp = psSc.tile([128, S], f32, tag="sc", name="t1_ps")
                for m in range(MT):
                    nc.tensor.transpose(tp[:, 128 * m:128 * (m + 1)],
                                        h1f[:, m, 128 * c:128 * (c + 1)],
                                        identf[:])
                nc.vector.tensor_copy(t1[:, c, :], tp[:])

            # -------- qkv: V, then K^T head 0 + AG0, Q, K^T heads 1-3 -------
            agkv_in = [dram.tile([2 * DH, S], bf16, name=f"agkvi{h}")
                       for h in range(HEADS)]
            agkv_out = [dram.tile([NCORES * 2 * DH, S], bf16, name=f"agkvo{h}",
                                  addr_space="Shared")
                        for h in range(HEADS)]
            vv = apool.tile([128, MT, H], bf16, tag="vvqt", name="vv_sb")
            for m in range(MT):
                ps = psAgg.tile([128, H], f32, tag="agg", name="v_ps")
                for c in range(H // 128):
                    nc.tensor.matmul(ps[:], t1[:, c, 128 * m:128 * (m + 1)],
                                     inwT[:, c, 2 * H:3 * H], start=(c == 0),
                                     stop=(c == H // 128 - 1))
                nc.vector.tensor_tensor(vv[:, m, :], ps[:], inbvbc[:], op=ALU.add)

            kt = apool.tile([128, HEADS, S], bf16, tag="kt", name="kt_sb")

            def k_head(h):
                ps2 = psAgg.tile([128, S], f32, tag="agg", name="k_ps")
                for c in range(H // 128):
                    nc.tensor.matmul(ps2[:],
                                     inwT[:, c, H + 128 * h:H + 128 * (h + 1)],
                                     t1[:, c, :], start=(c == 0),
                                     stop=(c == H // 128 - 1))
                nc.vector.tensor_scalar_add(kt[:, h, :], ps2[:], inbk[:, h:h + 1])
                nc.sync.dma_start(agkv_in[h][0:DH, :], kt[:, h, :])
                nc.sync.dma_start(
                    agkv_in[h][DH:2 * DH, :].rearrange("p (m v) -> p m v", m=MT),
                    vv[:, :, 128 * h:128 * (h + 1)])
                nc.gpsimd.collective_compute(
                    "AllGather", ALU.bypass, ins=[agkv_in[h].opt()],
                    outs=[agkv_out[h].opt()],
                    replica_groups=[list(range(NCORES))])

            k_head(0)

            qt = apool.tile([128, HEADS, S], bf16, tag="vvqt", name="qt_sb")
            inbq_s = cpool.tile([128, HEADS], f32, tag="inbq_s", name="inbq_s")
            nc.vector.tensor_scalar_mul(inbq_s[:], inbq[:], SCALE)
            for h in range(HEADS):
                ps = psAgg.tile([128, S], f32, tag="agg", name="q_ps")
                for c in range(H // 128):
                    nc.tensor.matmul(ps[:], inwT[:, c, 128 * h:128 * (h + 1)],
                                     t1[:, c, :], start=(c == 0),
                                     stop=(c == H // 128 - 1))
                nc.vector.tensor_scalar(qt[:, h, :], ps[:], SCALE,
                                        inbq_s[:, h:h + 1], op0=ALU.mult,
                                        op1=ALU.add)
            for h in range(1, HEADS):
                k_head(h)

            # -------- attention, heads pipelined over their AllGathers ------
            oT = apool.tile([128, HEADS, S], bf16, tag="oT", name="oT_sb")
            pending_epi = []

            def head_epilogue(h, av_ps, sum_ps):
                rsum = wpool.tile([1, S], f32, tag="rsum", name="rsum_t",
                                  bufs=2)
                nc.vector.reciprocal(rsum[:], sum_ps[:])
                bc_ps = psSc.tile([128, S], f32, tag="sc", name="bc_ps")
                nc.tensor.matmul(bc_ps[:], ones_row[:], rsum[:], start=True,
                                 stop=True)
                bc_sb = wpool.tile([128, S], f32, tag="bc_sb", name="bc_sb",
                                   bufs=1)
                nc.vector.tensor_copy(bc_sb[:], bc_ps[:])
                nc.vector.tensor_tensor(oT[:, h, :], av_ps[:], bc_sb[:],
                                        op=ALU.mult)

            for h in range(HEADS):
                agv = agkv_out[h][:].rearrange("(c z p) s -> p z c s", p=128, z=2)
                kth = kvpool.tile([128, NCORES, S], bf16, tag="kth", name="kth")
                nc.sync.dma_start(kth[:], agv[:, 0, :, :])
                vh = kvpool.tile([128, NT, DH], bf16, tag="vh", name="vh")
                nc.sync.dma_start(
                    vh[:].rearrange("p (c t) v -> p c t v", c=NCORES),
                    agv[:, 1, :, :].rearrange("p c (t v) -> p c t v", t=MT))
                kthf = kth[:].rearrange("p c s -> p (c s)")
                av_ps = psAv.tile([128, S], f32, tag="av", name="av_ps")
                sum_ps = psSum.tile([1, S], f32, tag="sum", name="sum_ps")
                l1_prev = None
                deferred = []
                for g in range(NT // 2):            # 2 key blocks per group
                    sca = psSc.tile([128, 2, S], f32, tag="sc", name="sca_ps")
                    for j in range(2):
                        b = 2 * g + j
                        nc.tensor.matmul(sca[:, j, :],
                                         kthf[:, 128 * b:128 * (b + 1)],
                                         qt[:, h, :], start=True, stop=True)
                    ex = wpool.tile([128, 2, S], bf16, tag="ex", name="ex_t", bufs=2)
                    nc.scalar.activation(ex[:], sca[:], AF.Exp)

                    def tail(g=g, ex=ex):
                        nonlocal l1_prev
                        for j in range(2):
                            nc.tensor.matmul(av_ps[:], vh[:, 2 * g + j, :],
                                             ex[:, j, :],
                                             start=(g == 0 and j == 0),
                                             stop=(g == NT // 2 - 1 and j == 1))
                        l1 = wpool.tile([128, S], bf16, tag="l1", name="l1_t")
                        nc.vector.tensor_tensor(l1[:], ex[:, 0, :], ex[:, 1, :],
                                                op=ALU.add)
                        if g % 2 == 0:
                            l1_prev = l1
                        else:
                            l2 = wpool.tile([128, S], bf16, tag="l2",
                                            name="l2_t", bufs=1)
                            nc.vector.tensor_tensor(l2[:], l1_prev[:], l1[:],
                                                    op=ALU.add)
                            nc.tensor.matmul(sum_ps[:], ones_bf[:], l2[:],
                                             start=(g == 1),
                                             stop=(g == NT // 2 - 1))
                    if g < 2:
                        deferred.append(tail)
                    else:
                        if g == 2:
                            if pending_epi:
                                pending_epi.pop()()
                            for fn in deferred:
                                fn()
                            deferred = []
                        tail()
                pending_epi.append(
                    lambda h=h, a=av_ps, s=sum_ps: head_epilogue(h, a, s))
            pending_epi.pop()()

            # -------- M = attn @ out_w.T ; H2 = h1 + M + out_b --------------
            h2 = apool.tile([128, MT, H], bf16, tag="h23", name="h2_sb")
            for m in range(MT):
                ps = psAgg.tile([128, H], f32, tag="agg", name="m_ps")
                for c in range(HEADS):
                    nc.tensor.matmul(ps[:], oT[:, c, 128 * m:128 * (m + 1)],
                                     outwT[:, c, :], start=(c == 0),
                                     stop=(c == HEADS - 1))
                t = wpool.tile([128, H], f32, tag="ep2", name="ep2_t")
                nc.vector.tensor_tensor(t[:], ps[:], h1f[:, m, :], op=ALU.add)
                nc.vector.tensor_tensor(h2[:, m, :], t[:], outbbc[:], op=ALU.add)

            # -------- T2, P2 (unscaled), AllGather halves -------------------
            t2 = apool.tile([128, H // 128, S], bf16, tag="t23", name="t2_sb")
            for c in range(H // 128):
                tp = psSc.tile([128, S], bf16, tag="sc", name="t2_ps")
                for m in range(MT):
                    nc.tensor.transpose(tp[:, 128 * m:128 * (m + 1)],
                                        h2[:, m, 128 * c:128 * (c + 1)], ident[:])
                nc.vector.tensor_copy(t2[:, c, :], tp[:])
            ag2_in = [dram.tile([2 * 128, H], bf16, name=f"ag2i{hf}")
                      for hf in range(2)]
            ag2_out = [dram.tile([NCORES * 2 * 128, H], bf16, name=f"ag2o{hf}",
                                 addr_space="Shared")
                       for hf in range(2)]
            for half in range(2):
                for i in range(2):
                    m = 2 * half + i
                    ps = psAgg.tile([128, H], f32, tag="agg", name="p2_ps")
                    for c in range(H // 128):
                        nc.tensor.matmul(ps[:], t2[:, c, 128 * m:128 * (m + 1)],
                                         w2[:, c, :], start=(c == 0),
                                         stop=(c == H // 128 - 1))
                    p2c = wpool.tile([128, H], bf16, tag="p2c", name="p2c_t")
                    nc.scalar.activation(p2c[:], ps[:], AF.Identity,
                                         scale=dinv_col[:, m:m + 1])
                    nc.sync.dma_start(
                        ag2_in[half][:].rearrange("(mm p) h -> p mm h",
                                                  p=128)[:, i, :],
                        p2c[:])
                nc.gpsimd.collective_compute(
                    "AllGather", ALU.bypass, ins=[ag2_in[half].opt()],
                    outs=[ag2_out[half].opt()],
                    replica_groups=[list(range(NCORES))])

            # -------- conv2: single pass, rhs streamed from the AG halves ---
            h3 = apool.tile([128, MT, H], bf16, tag="h23", name="h3_sb")
            hps2 = [psAgg.tile([128, H], f32, tag="agg", name=f"cv2{i}")
                    for i in range(2)]
            hps2 += [psSc.tile([128, H], f32, tag="sc", name=f"cv2{i + 2}")
                     for i in range(2)]
            for half in range(2):
                agv2 = ag2_out[half][:].rearrange("(cc p) h -> p cc h", p=128)
                for cc in range(NCORES):
                    rhs = spool.tile([128, 2, H], bf16, tag="agc", name="agc_t")
                    nc.sync.dma_start(rhs[:], agv2[:, 2 * cc:2 * (cc + 1), :])
                    for i in range(2):
                        for m in range(MT):
                            nc.tensor.matmul(
                                hps2[m][:],
                                acT[:, 4 * cc + 2 * half + i,
                                    128 * m:128 * (m + 1)],
                                rhs[:, i, :],
                                start=(half == 0 and cc == 0 and i == 0),
                                stop=(half == 1 and cc == NCORES - 1 and i == 1))
            for m in range(MT):
                t = wpool.tile([128, H], f32, tag="ep1", name="ep3_t")
                nc.scalar.activation(t[:], hps2[m][:], AF.Identity,
                                     scale=dinv_col[:, m:m + 1])
                nc.vector.tensor_tensor(t[:], t[:], b2bc[:], op=ALU.add)
                nc.vector.tensor_scalar_max(h3[:, m, :], t[:], 0.0)

            t3 = apool.tile([128, H // 128, S], bf16, tag="t23", name="t3_sb")
            for c in range(H // 128):
                tp = psSc.tile([128, S], bf16, tag="sc", name="t3_ps")
                for m in range(MT):
                    nc.tensor.transpose(tp[:, 128 * m:128 * (m + 1)],
                                        h3[:, m, 128 * c:128 * (c + 1)], ident[:])
                nc.vector.tensor_copy(t3[:, c, :], tp[:])

            for m in range(MT):
                ps = psAv.tile([128, C], f32, tag="av", name="oc_ps")
                for c in range(H // 128):
                    nc.tensor.matmul(ps[:], t3[:, c, 128 * m:128 * (m + 1)],
                                     wc[:, c, :], start=(c == 0),
                                     stop=(c == H // 128 - 1))
                ot = wpool.tile([128, C], f32, tag="ot", name="ot_t", bufs=1)
                nc.vector.tensor_tensor(ot[:], ps[:], bccbc[:], op=ALU.add)
                nc.sync.dma_start(
                    out_t.ap().rearrange("(mm p) c -> p mm c", p=128)[:, m, :],
                    ot[:])

    nc.compile()
    return nc


def _get_compiled():
    global _compiled
    if _compiled is None:
        _compiled = _build()
    return _compiled


def _prep_inputs(x, edge_index, W1, b1, in_w, in_b, out_w, out_b, W2, b2, Wc, bc):
    x = np.asarray(x, np.float32)
    ei = np.asarray(edge_index, np.int64)
    src, dst = ei[0], ei[1]

    shared = {
        "xT": np.ascontiguousarray(x.T).astype(BF16),
        "w1": np.asarray(W1, np.float32).astype(BF16),
        "inwT": np.ascontiguousarray(np.asarray(in_w, np.float32).T).astype(BF16),
        "outwT": np.ascontiguousarray(np.asarray(out_w, np.float32).T).astype(BF16),
        "w2": np.asarray(W2, np.float32).astype(BF16),
        "wc": np.asarray(Wc, np.float32).astype(BF16),
        "b1bc": np.tile(np.asarray(b1, np.float32)[None, :], (128, 1)),
        "b2bc": np.tile(np.asarray(b2, np.float32)[None, :], (128, 1)),
        "outbbc": np.tile(np.asarray(out_b, np.float32)[None, :], (128, 1)),
        "inbvbc": np.tile(np.asarray(in_b, np.float32)[2 * H:3 * H][None, :],
                          (128, 1)),
        "bccbc": np.tile(np.asarray(bc, np.float32)[None, :], (128, 1)),
        "inbq": np.ascontiguousarray(
            np.asarray(in_b, np.float32)[0:H].reshape(HEADS, 128).T),
        "inbk": np.ascontiguousarray(
            np.asarray(in_b, np.float32)[H:2 * H].reshape(HEADS, 128).T),
        "ident": np.eye(128, dtype=np.float32).astype(BF16),
    }

    in_maps = []
    for k in range(NCORES):
        lo, hi = S * k, S * (k + 1)
        sel = (dst >= lo) & (dst < hi)
        ac = np.zeros((N, S), np.float32)
        np.add.at(ac, (src[sel], dst[sel] - lo), 1.0)
        ac[np.arange(lo, hi), np.arange(S)] += 1.0  # self loops
        m = dict(shared)
        m["acT"] = ac.astype(BF16)
        in_maps.append(m)
    return in_maps


def _run_device(in_maps):
    from concourse import bass_utils
    nc = _get_compiled()
    res = bass_utils.run_bass_kernel_spmd(nc, in_maps,
                                          core_ids=list(range(NCORES)))
    return np.concatenate([res.results[k]["y"] for k in range(NCORES)], axis=0)


def _run_subprocess(in_maps):
    """Fresh-process retry: survives a wedged NRT/axon client in this one."""
    import os
    import subprocess
    import tempfile
    d = tempfile.mkdtemp(prefix="gcnkrn_")
    fin = os.path.join(d, "in.npz")
    fout = os.path.join(d, "out.npy")
    np.savez(fin, **{f"c{k}_{n}": a for k, m in enumerate(in_maps)
                     for n, a in m.items()})
    env = {k: v for k, v in os.environ.items() if k != "JAX_PLATFORMS"}
    subprocess.run([sys.executable, os.path.abspath(__file__), fin, fout],
                   check=True, timeout=900, env=env)
    return np.load(fout)


def kernel(**inputs) -> np.ndarray:
    in_maps = _prep_inputs(**inputs)
    try:
        return _run_device(in_maps)
    except Exception:
        err = None
        for _ in range(2):
            try:
                return _run_subprocess(in_maps)
            except Exception as e:
                err = e
        raise err


if __name__ == "__main__":
    _fin, _fout = sys.argv[1], sys.argv[2]
    _z = np.load(_fin)
    _maps = [{} for _ in range(NCORES)]
    _bf_names = {"acT", "xT", "w1", "inwT", "outwT", "w2", "wc", "ident"}
    for _key in _z.files:
        _ck, _name = _key.split("_", 1)
        _a = _z[_key]
        if _a.dtype == np.dtype("V2") or _name in _bf_names:
            _a = _a.view(BF16) if _a.dtype.itemsize == 2 else _a.astype(BF16)
        _maps[int(_ck[1:])][_name] = _a
    np.save(_fout, _run_device(_maps))
